# revision 1
# baseline (speedup 1.0000x reference)
"""MultiHeadLinearAttention Trainium2 Bass kernel — 8-core SPMD.

Problem (per reference):
  q = elu(LN(Xq @ Wq.T + bq)) + 1 ; k = elu(LN(Xk @ Wk.T + bk)) + 1
  v = Xv @ Wv.T + bv
  kv = sum_n k[n] (x) v[n]   (per head, [D,D]);  ksum = sum_n k[n]
  out = ((q @ kv) / (q . ksum + 1e-8)) @ Wo.T + bo

Sharding: core c -> batch b = c//2, token half h = c%2 (2048 q AND k/v
tokens each). Per-pair (cores 2b, 2b+1) AllReduce of kv/ksum partials
(~266 KB) completes the sum over all 4096 k/v tokens of the batch.

Layouts on chip (per core):
  k,v: [tok x feat] (LN over free dim; kv contraction over token partitions)
  q:   [feat x tok] (q^T feeds num = kv_bd^T @ q^T and out-proj lhsT)
LayerNorm mean is folded into the weights on host (W~ = W^T(I-J/E),
b~ = b - mean(b)); gq/gk==1, betaq/betak==0 (asserted) so
LN(y) = (y - mu(y)) * rsqrt(var + eps) = u * exp(-0.5*ln(mean(u^2)+eps)).
elu(z)+1 = exp(min(z,0)) + relu(z).

All matmuls run as float32r (FP32 bits read at FP22 precision, full PE
rate at moving-dim >= 256).
"""

import os

import numpy as np

B, NSEQ, E, H, D = 4, 4096, 1024, 16, 64
NCORES = 8
T = NSEQ // 2          # tokens per core
TT = T // 128          # token tiles (16)
EI = E // 128          # feature tiles (8)
LN_EPS = 1e-5

_NC_CACHE = {}


def _build_nc(dbg=False):
    from concourse import bacc
    import concourse.bass as bass
    import concourse.mybir as mybir
    import concourse.tile as tile

    f32 = mybir.dt.float32
    f32r = mybir.dt.float32r
    Alu = mybir.AluOpType
    Act = mybir.ActivationFunctionType
    RG = [[0, 1], [2, 3], [4, 5], [6, 7]]

    def r(ap):
        return ap.bitcast(f32r)

    nc = bacc.Bacc(num_devices=NCORES)

    xqT = nc.dram_tensor("xqT", [E, T], f32r, kind="ExternalInput")
    xkT = nc.dram_tensor("xkT", [E, T], f32r, kind="ExternalInput")
    xvT = nc.dram_tensor("xvT", [E, T], f32r, kind="ExternalInput")
    wqT = nc.dram_tensor("wqT", [E, E], f32r, kind="ExternalInput")
    wkT = nc.dram_tensor("wkT", [E, E], f32r, kind="ExternalInput")
    wvT = nc.dram_tensor("wvT", [E, E], f32r, kind="ExternalInput")
    woT = nc.dram_tensor("woT", [E, E], f32r, kind="ExternalInput")
    bq2d = nc.dram_tensor("bq2d", [128, EI], f32, kind="ExternalInput")
    bkR = nc.dram_tensor("bkR", [1, E], f32r, kind="ExternalInput")
    bvR = nc.dram_tensor("bvR", [1, E], f32r, kind="ExternalInput")
    boR = nc.dram_tensor("boR", [1, E], f32r, kind="ExternalInput")
    onesR = nc.dram_tensor("onesR", [1, 128], f32r, kind="ExternalInput")
    onesC = nc.dram_tensor("onesC", [128, 1], f32r, kind="ExternalInput")
    zerosBD = nc.dram_tensor("zerosBD", [128, E], f32r, kind="ExternalInput")
    out_d = nc.dram_tensor("out", [T, E], f32, kind="ExternalOutput")
    if dbg:
        dbg_ar = nc.dram_tensor("dbg_ar", [128, 520], f32, kind="ExternalOutput")
        dbg_qf = nc.dram_tensor("dbg_qf", [128, T], f32, kind="ExternalOutput")
        dbg_num = nc.dram_tensor("dbg_num", [128, T], f32, kind="ExternalOutput")

    with tile.TileContext(nc) as tc:
        with tc.tile_pool(name="const", bufs=1) as cp, \
             tc.tile_pool(name="dram", bufs=1, space="DRAM") as dp:
            ones_col = cp.tile([128, 1], f32, tag="ones_col")
            nc.vector.memset(ones_col, 1.0)
            onesR_sb = cp.tile([1, 128], f32r, tag="onesR_sb")
            nc.sync.dma_start(out=onesR_sb, in_=onesR[:, :])
            onesC_sb = cp.tile([128, 1], f32r, tag="onesC_sb")
            nc.sync.dma_start(out=onesC_sb, in_=onesC[:, :])
            zrow_sb = cp.tile([1, E], f32r, tag="zrow_sb")
            nc.sync.dma_start(out=zrow_sb, in_=zerosBD[0:1, :])
            eps_sb = cp.tile([128, 1], f32, tag="eps_sb")
            nc.vector.memset(eps_sb, LN_EPS)
            eps8_sb = cp.tile([128, 1], f32, tag="eps8_sb")
            nc.vector.memset(eps8_sb, 1e-8)
            bq_sb = cp.tile([128, EI], f32, tag="bq_sb")
            nc.sync.dma_start(out=bq_sb, in_=bq2d[:, :])
            bk_sb = cp.tile([1, E], f32r, tag="bk_sb")
            nc.sync.dma_start(out=bk_sb, in_=bkR[:, :])
            bv_sb = cp.tile([1, E], f32r, tag="bv_sb")
            nc.sync.dma_start(out=bv_sb, in_=bvR[:, :])
            bo_sb = cp.tile([1, E], f32r, tag="bo_sb")
            nc.sync.dma_start(out=bo_sb, in_=boR[:, :])
            bk_b = cp.tile([128, E], f32r, tag="bk_b")
            nc.sync.dma_start(out=bk_b, in_=bkR[:, :].to_broadcast([128, E]))
            bv_b = cp.tile([128, E], f32r, tag="bv_b")
            nc.sync.dma_start(out=bv_b, in_=bvR[:, :].to_broadcast([128, E]))
            bo_b = cp.tile([128, E], f32r, tag="bo_b")
            nc.sync.dma_start(out=bo_b, in_=boR[:, :].to_broadcast([128, E]))
            kvbd = cp.tile([128, E], f32r, tag="kvbd")
            ar_sb = cp.tile([128, 520], f32, tag="ar_sb")
            cc_in = dp.tile([128, 520], f32, tag="cc_in")
            cc_out = dp.tile([128, 520], f32, tag="cc_out")
            rstd_d = dp.tile([1, T], f32, tag="rstd_d")

            xkT_v = xkT.rearrange("(i p) n -> p i n", p=128)
            xvT_v = xvT.rearrange("(i p) n -> p i n", p=128)

            # ============ Phase A: k/v proj + elu + kv, two half-passes =
            with tc.tile_pool(name="pares", bufs=1) as pares, \
                 tc.tile_pool(name="pa1w", bufs=1) as pa1w, \
                 tc.tile_pool(name="pa2", bufs=2) as pa2, \
                 tc.tile_pool(name="pskv", bufs=1, space="PSUM") as pskv:
                wk_sb = pa1w.tile([128, EI, E], f32r, tag="wk")
                wv_sb = pa1w.tile([128, EI, E], f32r, tag="wv")
                wkT_v = wkT.rearrange("(i p) j -> p i j", p=128)
                wvT_v = wvT.rearrange("(i p) j -> p i j", p=128)
                for i in range(EI):
                    nc.sync.dma_start(out=wk_sb[:, i, :], in_=wkT_v[:, i, :])
                    nc.sync.dma_start(out=wv_sb[:, i, :], in_=wvT_v[:, i, :])
                ss_all = pa1w.tile([128, TT], f32, tag="ss_all")
                rstd_all = pa1w.tile([128, TT], f32, tag="rstd_all")

                kv_ps = [pskv.tile([128, 512], f32, tag=f"kv{q}",
                                   name=f"kv{q}") for q in range(4)]
                ksum_ps = pskv.tile([128, 8], f32, tag="ksum")
                for q in range(4):
                    nc.tensor.matmul(kv_ps[q], onesR_sb[:, :],
                                     zrow_sb[:, 0:512], start=True,
                                     stop=False, skip_group_check=True)
                nc.tensor.matmul(ksum_ps, onesR_sb[:, :], zrow_sb[:, 0:8],
                                 start=True, stop=False, skip_group_check=True)

                HT = TT // 2
                for half in range(2):
                    ku_t = {}
                    vu_t = {}
                    with tc.tile_pool(name=f"pa_{half}", bufs=2) as pa, \
                         tc.tile_pool(name=f"psa_{half}", bufs=1,
                                      space="PSUM") as psa1:
                        for t in range(half * HT, half * HT + HT):
                            ts = slice(128 * t, 128 * t + 128)
                            xk = pa.tile([128, EI, 128], f32r, tag="xk")
                            nc.sync.dma_start(out=xk, in_=xkT_v[:, :, ts])
                            xv = pa.tile([128, EI, 128], f32r, tag="xv")
                            nc.sync.dma_start(out=xv, in_=xvT_v[:, :, ts])

                            k_ps = psa1.tile([128, E], f32, tag="kps")
                            for i in range(EI):
                                for jh in range(2):
                                    js = slice(512 * jh, 512 * jh + 512)
                                    nc.tensor.matmul(
                                        k_ps[:, js], xk[:, i, :],
                                        wk_sb[:, i, js],
                                        start=(i == 0), stop=(i == EI - 1))
                            ku = pares.tile([128, E], f32, tag=f"ku{t % HT}",
                                            name=f"ku{t}")
                            nc.vector.scalar_tensor_tensor(
                                out=ku, in0=k_ps, scalar=1.0,
                                in1=bk_b.bitcast(f32), op0=Alu.mult,
                                op1=Alu.add)
                            scrap = pa.tile([128, E], f32, tag="scrap")
                            nc.scalar.activation(out=scrap, in_=ku,
                                                 func=Act.Square,
                                                 accum_out=ss_all[:, t:t + 1])
                            ku_t[t] = ku

                            vu = pares.tile([128, E], f32r, tag=f"vu{t % HT}",
                                            name=f"vu{t}")
                            for jh in range(2):
                                js = slice(512 * jh, 512 * jh + 512)
                                v_ps = psa1.tile([128, 512], f32, tag="vps")
                                for i in range(EI):
                                    nc.tensor.matmul(
                                        v_ps, xv[:, i, :], wv_sb[:, i, js],
                                        start=(i == 0), stop=(i == EI - 1))
                                nc.vector.scalar_tensor_tensor(
                                    out=vu[:, js], in0=v_ps, scalar=1.0,
                                    in1=bv_b.bitcast(f32)[:, js],
                                    op0=Alu.mult, op1=Alu.add)
                            vu_t[t] = vu

                    # batched rstd for this half (one Ln/Exp table trip)
                    hsl = slice(half * HT, half * HT + HT)
                    nc.scalar.activation(out=rstd_all[:, hsl],
                                         in_=ss_all[:, hsl], func=Act.Ln,
                                         scale=1.0 / E, bias=eps_sb)
                    nc.scalar.activation(out=rstd_all[:, hsl],
                                         in_=rstd_all[:, hsl], func=Act.Exp,
                                         scale=-0.5)

                    for t in range(half * HT, half * HT + HT):
                        rs = rstd_all[:, t:t + 1]
                        km = pa2.tile([128, E], f32, tag="km")
                        nc.vector.tensor_scalar(out=km, in0=ku_t[t],
                                                scalar1=rs, scalar2=0.0,
                                                op0=Alu.mult, op1=Alu.min)
                        kf = pa2.tile([128, E], f32r, tag="kf")
                        nc.scalar.activation(out=kf, in_=ku_t[t],
                                             func=Act.Relu, scale=rs)
                        nc.scalar.activation(out=km, in_=km, func=Act.Exp)
                        nc.vector.tensor_tensor(out=kf, in0=kf.bitcast(f32),
                                                in1=km, op=Alu.add)
                        for q4 in range(4):
                            vq = vu_t[t][:, 256 * q4:256 * q4 + 256]
                            for hf in range(2):
                                pr = 2 * q4 + hf
                                kp = kf[:, 128 * pr:128 * pr + 128]
                                nc.tensor.matmul(
                                    kv_ps[q4][:, 256 * hf:256 * hf + 256],
                                    kp, vq, start=False, stop=(t == TT - 1),
                                    skip_group_check=True)
                                nc.tensor.matmul(
                                    ksum_ps[:, pr:pr + 1], kp.bitcast(f32),
                                    ones_col[:, :], start=False,
                                    stop=(t == TT - 1), skip_group_check=True)

                pack = pa1w.tile([128, 520], f32, tag="pack")
                for p in range(8):
                    q4, odd = divmod(p, 2)
                    c = 64 * p
                    if odd == 0:
                        nc.vector.tensor_copy(out=pack[0:64, c:c + 64],
                                              in_=kv_ps[q4][0:64, 0:64])
                        nc.vector.tensor_copy(out=pack[64:128, c:c + 64],
                                              in_=kv_ps[q4][64:128, 64:128])
                    else:
                        nc.vector.tensor_copy(out=pack[0:64, c:c + 64],
                                              in_=kv_ps[q4][0:64, 384:448])
                        nc.vector.tensor_copy(out=pack[64:128, c:c + 64],
                                              in_=kv_ps[q4][64:128, 448:512])
                nc.vector.tensor_copy(out=pack[:, 512:520], in_=ksum_ps[:, :])
                nc.sync.dma_start(out=cc_in, in_=pack)

            nc.gpsimd.collective_compute(
                "AllReduce", Alu.add, replica_groups=RG,
                ins=[cc_in[:, :]], outs=[cc_out[:, :]])
            nc.sync.dma_start(out=ar_sb, in_=cc_out[:, :])
            if dbg:
                nc.sync.dma_start(out=dbg_ar[:, :], in_=ar_sb)

            # block-diagonal kv + per-head ksum lhsT
            nc.sync.dma_start(out=kvbd, in_=zerosBD[:, :])
            ev_dst = kvbd[0:64, :].rearrange("p (a two c) -> p a two c",
                                             two=2, c=64)[:, :, 0, :]
            nc.vector.tensor_copy(
                out=ev_dst,
                in_=ar_sb[0:64, 0:512].rearrange("p (a c) -> p a c", c=64))
            od_dst = kvbd[64:128, :].rearrange("p (a two c) -> p a two c",
                                               two=2, c=64)[:, :, 1, :]
            nc.vector.tensor_copy(
                out=od_dst,
                in_=ar_sb[64:128, 0:512].rearrange("p (a c) -> p a c", c=64))
            ksum2 = cp.tile([128, 16], f32r, tag="ksum2")
            nc.sync.dma_start(out=ksum2, in_=zerosBD[:, 0:16])
            for jj in range(EI):
                nc.vector.tensor_copy(out=ksum2[0:64, 2 * jj:2 * jj + 1],
                                      in_=ar_sb[0:64, 512 + jj:513 + jj])
                nc.vector.tensor_copy(out=ksum2[64:128, 2 * jj + 1:2 * jj + 2],
                                      in_=ar_sb[64:128, 512 + jj:513 + jj])

            # ============ Phase B1: q projection + LN stats =============
            with tc.tile_pool(name="pu", bufs=1) as pu:
                u_t = [pu.tile([128, T], f32r, tag=f"u{j}", name=f"u{j}")
                       for j in range(EI)]
                rstd_b = pu.tile([128, T], f32, tag="rstd_b")

                with tc.tile_pool(name="pb1", bufs=1) as pb1, \
                     tc.tile_pool(name="pbw", bufs=16) as pbw, \
                     tc.tile_pool(name="pbs", bufs=2) as pbs, \
                     tc.tile_pool(name="psb1", bufs=2, space="PSUM") as psb1:
                    xq = pb1.tile([128, EI, T], f32r, tag="xq")
                    xqT_v = xqT.rearrange("(i p) n -> p i n", p=128)
                    for i in range(EI):
                        nc.sync.dma_start(out=xq[:, i, :], in_=xqT_v[:, i, :])
                    ssq_ps = psb1.tile([1, T], f32, tag="ssq", bufs=1)
                    rstd_row = pbs.tile([1, T], f32, tag="rstd_row", bufs=1)
                    for j in range(EI):
                        wq_j = [pbw.tile([128, 128], f32r, tag="wqt",
                                         name=f"wq{j}_{i}") for i in range(EI)]
                        for i in range(EI):
                            nc.sync.dma_start(
                                out=wq_j[i],
                                in_=wqT[128 * i:128 * i + 128,
                                        128 * j:128 * j + 128])
                        usq = pbs.tile([128, T], f32r, tag="usq")
                        for sh in range(2):
                            q_ps = psb1.tile([128, 1024], f32, tag="qps")
                            for i in range(EI):
                                for sq in range(2):
                                    sl = slice(512 * sq, 512 * sq + 512)
                                    gl = slice(1024 * sh + 512 * sq,
                                               1024 * sh + 512 * sq + 512)
                                    nc.tensor.matmul(
                                        q_ps[:, sl], wq_j[i], xq[:, i, gl],
                                        start=(i == 0), stop=(i == EI - 1))
                            hs = slice(1024 * sh, 1024 * sh + 1024)
                            nc.scalar.activation(out=usq[:, hs], in_=q_ps,
                                                 func=Act.Square,
                                                 bias=bq_sb[:, j:j + 1])
                            nc.vector.tensor_scalar_add(
                                out=u_t[j][:, hs], in0=q_ps,
                                scalar1=bq_sb[:, j:j + 1])
                            for sq in range(2):
                                gl = slice(1024 * sh + 512 * sq,
                                           1024 * sh + 512 * sq + 512)
                                nc.tensor.matmul(
                                    ssq_ps[:, gl], onesC_sb[:, :], usq[:, gl],
                                    start=(j == 0), stop=(j == EI - 1),
                                    skip_group_check=True)
                    nc.scalar.activation(out=rstd_row, in_=ssq_ps, func=Act.Ln,
                                         scale=1.0 / E, bias=eps_sb[0:1, :])
                    nc.scalar.activation(out=rstd_row, in_=rstd_row,
                                         func=Act.Exp, scale=-0.5)
                    nc.sync.dma_start(out=rstd_d, in_=rstd_row)
                    nc.sync.dma_start(out=rstd_b,
                                      in_=rstd_d.to_broadcast([128, T]))

                # ============ Phase B2: elu + per-head den + divide =====
                numT_t = u_t
                with tc.tile_pool(name="pb2", bufs=2) as pb2, \
                     tc.tile_pool(name="psb2", bufs=1, space="PSUM") as psb2:
                    den_ds = []
                    # B2a: elu feature map + per-head den for every pair
                    for j in range(EI):
                        qf = u_t[j]
                        nc.vector.tensor_tensor(out=qf, in0=qf.bitcast(f32),
                                                in1=rstd_b, op=Alu.mult)
                        m = pb2.tile([128, T], f32, tag="m", bufs=3)
                        nc.vector.tensor_scalar_min(out=m, in0=qf.bitcast(f32),
                                                    scalar1=0.0)
                        nc.scalar.activation(out=m, in_=m, func=Act.Exp)
                        nc.scalar.activation(out=qf, in_=qf.bitcast(f32),
                                             func=Act.Relu)
                        nc.vector.tensor_tensor(out=qf, in0=qf.bitcast(f32),
                                                in1=m, op=Alu.add)
                        den_d = dp.tile([2, T], f32, tag=f"dend{j}",
                                        name=f"dend{j}")
                        den_ps = psb2.tile([2, T], f32, tag="dps", bufs=1)
                        for s in range(4):
                            sl = slice(512 * s, 512 * s + 512)
                            nc.tensor.matmul(den_ps[:, sl],
                                             ksum2[:, 2 * j:2 * j + 2],
                                             qf[:, sl], start=True, stop=True)
                        den_sb = pb2.tile([2, T], f32, tag="den_sb", bufs=4)
                        nc.vector.tensor_copy(out=den_sb, in_=den_ps)
                        nc.gpsimd.dma_start(out=den_d, in_=den_sb)
                        den_ds.append(den_d)
                    # B2b: divide + num matmuls (bounces already in flight)
                    for j in range(EI):
                        qf = u_t[j]
                        den_d = den_ds[j]
                        den_b = pb2.tile([128, T], f32, tag="den_b", bufs=2)
                        nc.gpsimd.dma_start(
                            out=den_b,
                            in_=bass.AP(tensor=den_d.tensor,
                                        offset=den_d.offset,
                                        ap=[[T, 2], [0, 64], [1, T]]))
                        nc.vector.reciprocal_approx_fast(out=den_b, in_=den_b)
                        nc.vector.tensor_tensor(out=qf, in0=qf.bitcast(f32),
                                                in1=den_b, op=Alu.mult)
                        if dbg and j == 0:
                            nc.sync.dma_start(out=dbg_qf[:, :],
                                              in_=qf.bitcast(f32))
                        kv_j = kvbd[:, 128 * j:128 * j + 128]
                        for s in range(4):
                            sl = slice(512 * s, 512 * s + 512)
                            num_ps = psb2.tile([128, 512], f32, tag="nps",
                                               bufs=3)
                            nc.tensor.matmul(num_ps, kv_j, qf[:, sl],
                                             start=True, stop=True)
                            nc.scalar.activation(out=numT_t[j][:, sl],
                                                 in_=num_ps, func=Act.Copy)
                        if dbg and j == 0:
                            nc.sync.dma_start(out=dbg_num[:, :],
                                              in_=numT_t[0].bitcast(f32))


                # ============ Phase C: out projection ===============
                with tc.tile_pool(name="pc1", bufs=1) as pc1, \
                     tc.tile_pool(name="pc", bufs=2) as pcl, \
                     tc.tile_pool(name="psc", bufs=2, space="PSUM") as psc:
                    wo_sb = pc1.tile([128, EI, E], f32r, tag="wo")
                    nc.sync.dma_start(
                        out=wo_sb,
                        in_=woT.rearrange("(e p) j -> p e j", p=128))
                    for tt in range(TT):
                        tsl = slice(128 * tt, 128 * tt + 128)
                        o_ps = psc.tile([128, E], f32, tag="ops")
                        for e in range(EI):
                            lh = numT_t[e][:, tsl]
                            for jh in range(2):
                                js = slice(512 * jh, 512 * jh + 512)
                                nc.tensor.matmul(
                                    o_ps[:, js], lh, wo_sb[:, e, js],
                                    start=(e == 0), stop=(e == EI - 1))
                        o_sb = pcl.tile([128, E], f32, tag="osb")
                        nc.vector.scalar_tensor_tensor(
                            out=o_sb, in0=o_ps, scalar=1.0,
                            in1=bo_b.bitcast(f32), op0=Alu.mult, op1=Alu.add)
                        nc.sync.dma_start(out=out_d[tsl, :], in_=o_sb)

    nc.finalize()
    return nc


def _prep_inputs(inputs):
    """Host-side fold + per-core shard maps."""
    f = np.float32
    Wq, bq = inputs["Wq"], inputs["bq"]
    Wk, bk = inputs["Wk"], inputs["bk"]
    Wv, bv = inputs["Wv"], inputs["bv"]
    Wo, bo = inputs["Wo"], inputs["bo"]
    for name in ("gq", "gk"):
        assert np.allclose(np.asarray(inputs[name]), 1.0), f"{name} != 1 unsupported"
    for name in ("betaq", "betak"):
        assert np.allclose(np.asarray(inputs[name]), 0.0), f"{name} != 0 unsupported"

    wqT = np.ascontiguousarray(np.asarray(Wq, f).T)
    wqT = wqT - wqT.mean(axis=1, keepdims=True)
    bqf = np.asarray(bq, f) - np.asarray(bq, f).mean()
    wkT = np.ascontiguousarray(np.asarray(Wk, f).T)
    wkT = wkT - wkT.mean(axis=1, keepdims=True)
    bkf = np.asarray(bk, f) - np.asarray(bk, f).mean()
    wvT = np.ascontiguousarray(np.asarray(Wv, f).T)
    woT = np.ascontiguousarray(np.asarray(Wo, f).T)

    shared = {
        "wqT": np.ascontiguousarray(wqT, f),
        "wkT": np.ascontiguousarray(wkT, f),
        "wvT": wvT,
        "woT": woT,
        "bq2d": np.ascontiguousarray(bqf.reshape(EI, 128).T, f),
        "bkR": np.ascontiguousarray(bkf.reshape(1, E), f),
        "bvR": np.ascontiguousarray(np.asarray(bv, f).reshape(1, E)),
        "boR": np.ascontiguousarray(np.asarray(bo, f).reshape(1, E)),
        "onesR": np.ones((1, 128), f),
        "onesC": np.ones((128, 1), f),
        "zerosBD": np.zeros((128, E), f),
    }
    qe = np.asarray(inputs["query_embed"], f)
    ke = np.asarray(inputs["key_embed"], f)
    ve = np.asarray(inputs["value"], f)
    in_maps = []
    for c in range(NCORES):
        b, hh = divmod(c, 2)
        sl = slice(hh * T, (hh + 1) * T)
        m = dict(shared)
        m["xqT"] = np.ascontiguousarray(qe[b, sl, :].T)
        m["xkT"] = np.ascontiguousarray(ke[b, sl, :].T)
        m["xvT"] = np.ascontiguousarray(ve[b, sl, :].T)
        in_maps.append(m)
    return in_maps


def _run(inputs, trace=False):
    from concourse.bass_utils import run_bass_kernel_spmd

    dbg = bool(int(os.environ.get("KERNEL_DBG", "0")))
    key = "nc_dbg" if dbg else "nc"
    if key not in _NC_CACHE:
        _NC_CACHE[key] = _build_nc(dbg=dbg)
    nc = _NC_CACHE[key]
    in_maps = _prep_inputs(inputs)
    res = run_bass_kernel_spmd(nc, in_maps, core_ids=list(range(NCORES)),
                               trace=trace)
    out = np.empty((B, NSEQ, E), np.float32)
    for c in range(NCORES):
        b, hh = divmod(c, 2)
        out[b, hh * T:(hh + 1) * T, :] = res.results[c]["out"]
    return out, res


def kernel(**inputs):
    out, _ = _run(inputs, trace=False)
    return out


def kernel_traced(**inputs):
    """Like kernel() but also returns (exec_time_ns, trace_path)."""
    import sys, types
    try:
        import antenv
        if "antenv.axon_hooks" not in sys.modules:
            mod = types.ModuleType("antenv.axon_hooks")
            _h = [None]
            mod.set_axon_ntff_profile_hook = lambda h: _h.__setitem__(0, h)
            mod.get_axon_ntff_profile_hook = lambda: _h[0]
            sys.modules["antenv.axon_hooks"] = mod
            antenv.axon_hooks = mod
            from trn_agent_boot.trn_boot import _ntff_profile_via_ctypes
            mod.set_axon_ntff_profile_hook(
                _ntff_profile_via_ctypes("/opt/axon/libaxon_pjrt.so"))
    except Exception as e:  # profiling is best-effort
        print(f"NTFF hook setup failed: {e}")
    out, res = _run(inputs, trace=True)
    tp = res.instructions_and_trace[1] if res.instructions_and_trace else None
    return out, res.exec_time_ns, tp



# revision 18
# speedup vs baseline: 1.3555x; 1.3555x over previous
"""MultiHeadLinearAttention Trainium2 Bass kernel — 8-core SPMD.

Problem (per reference):
  q = elu(LN(Xq @ Wq.T + bq)) + 1 ; k = elu(LN(Xk @ Wk.T + bk)) + 1
  v = Xv @ Wv.T + bv
  kv = sum_n k[n] (x) v[n]   (per head, [D,D]);  ksum = sum_n k[n]
  out = ((q @ kv) / (q . ksum + 1e-8)) @ Wo.T + bo

Sharding: core c -> batch b = c//2, token half h = c%2 (2048 q AND k/v
tokens each). Per-pair (cores 2b, 2b+1) AllReduce of kv/ksum partials
(~266 KB) completes the sum over all 4096 k/v tokens of the batch.

Pipeline design (v2):
  Phase A (k/v): per-token-tile pipeline. rstd per tile via a DVE
    rsqrt bit-trick (no Ln/Exp -> no act-table switches), elu via
    min(exp(z),1)+relu(z) (exact), k_f/v in bf16 so the kv outer-
    product matmuls run 128-wide moving in bf16. kv/ksum accumulate
    in PSUM across all 16 tiles.
  Phase B1 (q proj) emitted between kv finish and AllReduce consume so
    the collective hides under q-projection matmuls.
  Phase B2+C: per-512-token-group pipeline. den is computed directly in
    broadcast form ([128,T]) via a replicated-ksum stationary, recip on
    DVE, divide fused into the num PSUM evacuation; out-projection for
    each group follows immediately so the PE never drains.

LayerNorm mean is folded into the weights on host (W~ = W^T(I-J/E),
b~ = b - mean(b)); gq/gk==1, betaq/betak==0 (asserted).
Projections run as float32r; attention-side matmuls run bf16.
"""

import os

import numpy as np

B, NSEQ, E, H, D = 4, 4096, 1024, 16, 64
NCORES = 8
T = NSEQ // 2          # tokens per core
TT = T // 128          # token tiles (16)
EI = E // 128          # feature tiles (8)
LN_EPS = 1e-5
RSQRT_MAGIC = 0x5F3759DF

_NC_CACHE = {}


def _build_nc():
    from concourse import bacc
    import concourse.bass as bass
    import concourse.mybir as mybir
    import concourse.tile as tile

    f32 = mybir.dt.float32
    f32r = mybir.dt.float32r
    bf16 = mybir.dt.bfloat16
    i32 = mybir.dt.int32
    Alu = mybir.AluOpType
    Act = mybir.ActivationFunctionType
    RG = [[0, 1], [2, 3], [4, 5], [6, 7]]

    nc = bacc.Bacc(num_devices=NCORES)

    xqT = nc.dram_tensor("xqT", [E, T], f32r, kind="ExternalInput")
    xkT = nc.dram_tensor("xkT", [E, T], f32r, kind="ExternalInput")
    xvT = nc.dram_tensor("xvT", [E, T], f32r, kind="ExternalInput")
    wqT = nc.dram_tensor("wqT", [E, E], f32r, kind="ExternalInput")
    wkT = nc.dram_tensor("wkT", [E, E], f32r, kind="ExternalInput")
    wvT = nc.dram_tensor("wvT", [E, E], f32r, kind="ExternalInput")
    woT = nc.dram_tensor("woT", [E, E], f32r, kind="ExternalInput")
    bq2d = nc.dram_tensor("bq2d", [128, EI], f32, kind="ExternalInput")
    bkR = nc.dram_tensor("bkR", [1, E], f32r, kind="ExternalInput")
    bvR = nc.dram_tensor("bvR", [1, E], f32r, kind="ExternalInput")
    boR = nc.dram_tensor("boR", [1, E], f32r, kind="ExternalInput")
    onesR = nc.dram_tensor("onesR", [1, 128], f32r, kind="ExternalInput")
    onesC = nc.dram_tensor("onesC", [128, 1], f32r, kind="ExternalInput")
    ident = nc.dram_tensor("ident", [128, 128], f32r, kind="ExternalInput")
    bones8_d = nc.dram_tensor("bones8", [8, 512], f32r, kind="ExternalInput")
    zerosBD = nc.dram_tensor("zerosBD", [128, E], f32r, kind="ExternalInput")
    out_d = nc.dram_tensor("out", [T, E], f32, kind="ExternalOutput")

    with tile.TileContext(nc) as tc:
        with tc.tile_pool(name="const", bufs=1) as cp, \
             tc.tile_pool(name="dram", bufs=1, space="DRAM") as dp:
            onesR_sb = cp.tile([1, 128], f32r, tag="onesR_sb")
            nc.sync.dma_start(out=onesR_sb, in_=onesR[:, :])
            onesC_sb = cp.tile([128, 1], f32r, tag="onesC_sb")
            nc.sync.dma_start(out=onesC_sb, in_=onesC[:, :])
            zrow_sb = cp.tile([1, E], f32r, tag="zrow_sb")
            nc.sync.dma_start(out=zrow_sb, in_=zerosBD[0:1, :])
            ones_col_bf = cp.tile([128, 1], bf16, tag="ones_col_bf")
            nc.vector.memset(ones_col_bf, 1.0)
            eps_sb = cp.tile([128, 1], f32, tag="eps_sb")
            nc.vector.memset(eps_sb, LN_EPS)
            bq_sb = cp.tile([128, EI], f32, tag="bq_sb")
            nc.sync.dma_start(out=bq_sb, in_=bq2d[:, :])
            bk_b = cp.tile([128, E], f32r, tag="bk_b")
            nc.sync.dma_start(out=bk_b, in_=bkR[:, :].to_broadcast([128, E]))
            bv_b = cp.tile([128, E], f32r, tag="bv_b")
            nc.sync.dma_start(out=bv_b, in_=bvR[:, :].to_broadcast([128, E]))
            bo_b = cp.tile([128, E], f32r, tag="bo_b")
            nc.sync.dma_start(out=bo_b, in_=boR[:, :].to_broadcast([128, E]))
            kvbd_bf = cp.tile([128, E], bf16, tag="kvbd_bf")
            ksumB = cp.tile([128, EI, 128], bf16, tag="ksumB")
            ksumT = cp.tile([8, 128], f32r, tag="ksumT")
            ident_sb = cp.tile([128, 128], f32r, tag="ident_sb")
            nc.sync.dma_start(out=ident_sb, in_=ident[:, :])
            bones8 = cp.tile([8, 512], f32r, tag="bones8")
            nc.sync.dma_start(out=bones8, in_=bones8_d[:, :])
            ar_sb = cp.tile([128, 520], f32, tag="ar_sb")
            cc_in = dp.tile([128, 520], f32, tag="cc_in")
            cc_out = dp.tile([128, 520], f32, tag="cc_out")

            ss_all = cp.tile([128, TT], f32, tag="ss_all")
            rs_all = cp.tile([128, TT], f32, tag="rs_all")
            rsq_s1 = cp.tile([128, TT], f32, tag="rsq_s1")
            rsq_t = cp.tile([128, TT], f32, tag="rsq_t")

            xkT_v = xkT.rearrange("(i p) n -> p i n", p=128)
            xvT_v = xvT.rearrange("(i p) n -> p i n", p=128)
            xqT_v = xqT.rearrange("(i p) n -> p i n", p=128)
            wkT_v = wkT.rearrange("(i p) j -> p i j", p=128)
            wvT_v = wvT.rearrange("(i p) j -> p i j", p=128)
            wqT_v = wqT.rearrange("(i p) j -> p i j", p=128)
            woT_v = woT.rearrange("(i p) j -> p i j", p=128)

            def rsqrt_batch(sl):
                """rs_all[:, sl] = rsqrt(ss_all[:, sl]/E + eps) via the
                inverse-sqrt bit trick + 2 Newton iterations (DVE only)."""
                s1 = rsq_s1[:, sl]
                nc.vector.tensor_scalar(out=s1, in0=ss_all[:, sl],
                                        scalar1=1.0 / E, scalar2=LN_EPS,
                                        op0=Alu.mult, op1=Alu.add)
                y = rs_all[:, sl]
                yi = y.bitcast(i32)
                nc.vector.tensor_scalar(out=yi, in0=s1.bitcast(i32),
                                        scalar1=1, scalar2=None,
                                        op0=Alu.logical_shift_right)
                nc.vector.tensor_scalar(out=yi, in0=yi,
                                        scalar1=RSQRT_MAGIC, scalar2=-1,
                                        op0=Alu.subtract, op1=Alu.mult)
                t_ = rsq_t[:, sl]
                for _ in range(2):
                    nc.vector.tensor_tensor(out=t_, in0=y, in1=y, op=Alu.mult)
                    nc.vector.tensor_tensor(out=t_, in0=t_, in1=s1,
                                            op=Alu.mult)
                    nc.vector.tensor_scalar(out=t_, in0=t_, scalar1=-0.5,
                                            scalar2=1.5, op0=Alu.mult,
                                            op1=Alu.add)
                    nc.vector.tensor_tensor(out=y, in0=y, in1=t_, op=Alu.mult)

            # ============ Phase A: k/v proj + elu + kv (16-tile pipe) ======
            with tc.tile_pool(name="paw", bufs=1) as paw, \
                 tc.tile_pool(name="pax", bufs=5) as pax, \
                 tc.tile_pool(name="pares", bufs=1) as pares, \
                 tc.tile_pool(name="pasc", bufs=3) as pasc, \
                 tc.tile_pool(name="pskv", bufs=1, space="PSUM") as pskv, \
                 tc.tile_pool(name="psproj", bufs=2, space="PSUM") as psproj:
                wk_sb = paw.tile([128, EI, E], f32r, tag="wk")
                wv_sb = paw.tile([128, EI, E], f32r, tag="wv")

                xk_t, xv_t = {}, {}

                def load_x(t):
                    tsl = slice(128 * t, 128 * t + 128)
                    xk = pax.tile([128, EI, 128], f32r, tag="xk",
                                  name=f"xk{t}")
                    nc.sync.dma_start(out=xk, in_=xkT_v[:, :, tsl])
                    xv = pax.tile([128, EI, 128], f32r, tag="xv",
                                  name=f"xv{t}")
                    nc.sync.dma_start(out=xv, in_=xvT_v[:, :, tsl])
                    xk_t[t], xv_t[t] = xk, xv

                # first x tiles before the bulk weight stream
                load_x(0)
                for i in range(EI):
                    nc.sync.dma_start(out=wk_sb[:, i, :], in_=wkT_v[:, i, :])
                    nc.sync.dma_start(out=wv_sb[:, i, :], in_=wvT_v[:, i, :])
                    if i < 3:
                        load_x(i + 1)

                kv_ps = pskv.tile([128, E], f32, tag="kv_ps")
                ksum_ps = pskv.tile([128, EI], f32, tag="ksum_ps")
                for h2 in range(2):
                    nc.tensor.matmul(kv_ps[:, 512 * h2:512 * h2 + 512],
                                     onesR_sb, zrow_sb[:, 0:512], start=True,
                                     stop=False, skip_group_check=True)
                nc.tensor.matmul(ksum_ps, onesR_sb, zrow_sb[:, 0:EI],
                                 start=True, stop=False, skip_group_check=True)

                ku_t, kf_t = {}, {}

                def elu_k(t):
                    rs = rs_all[:, t:t + 1]
                    kf = pares.tile([128, E], bf16, tag=f"kf{t % 3}",
                                    name=f"kf{t}")
                    nc.scalar.activation(out=kf, in_=ku_t[t], func=Act.Relu,
                                         scale=rs)
                    ee = pasc.tile([128, E], bf16, tag="ee")
                    nc.scalar.activation(out=ee, in_=ku_t[t], func=Act.Exp,
                                         scale=rs)
                    nc.vector.scalar_tensor_tensor(
                        out=kf, in0=ee, scalar=1.0, in1=kf,
                        op0=Alu.min, op1=Alu.add)
                    kf_t[t] = kf

                def kv_mms(t):
                    kf, vu = kf_t[t], xv_t[(t, "vu")]
                    for pr in range(EI):
                        kp = kf[:, 128 * pr:128 * pr + 128]
                        nc.tensor.matmul(
                            kv_ps[:, 128 * pr:128 * pr + 128], kp,
                            vu[:, 128 * pr:128 * pr + 128], start=False,
                            stop=(t == TT - 1), skip_group_check=True)
                        nc.tensor.matmul(
                            ksum_ps[:, pr:pr + 1], kp, ones_col_bf,
                            start=False, stop=(t == TT - 1),
                            skip_group_check=True)

                for t in range(TT):
                    if t >= 3 and t + 1 < TT:
                        load_x(t + 1)
                    xk, xv = xk_t[t], xv_t[t]
                    k_ps = psproj.tile([128, E], f32, tag="pps",
                                       name=f"kps{t}")
                    for i in range(EI):
                        for jh in range(2):
                            js = slice(512 * jh, 512 * jh + 512)
                            nc.tensor.matmul(k_ps[:, js], xk[:, i, :],
                                             wk_sb[:, i, js], start=(i == 0),
                                             stop=(i == EI - 1))
                    ku = pares.tile([128, E], f32, tag=f"ku{t % 8}",
                                    name=f"ku{t}")
                    nc.vector.scalar_tensor_tensor(
                        out=ku, in0=k_ps, scalar=1.0, in1=bk_b.bitcast(f32),
                        op0=Alu.mult, op1=Alu.add)
                    scrap = pasc.tile([128, E], bf16, tag="scrap")
                    nc.scalar.activation(out=scrap, in_=ku, func=Act.Square,
                                         accum_out=ss_all[:, t:t + 1])
                    ku_t[t] = ku

                    v_ps = psproj.tile([128, E], f32, tag="pps",
                                       name=f"vps{t}")
                    for i in range(EI):
                        for jh in range(2):
                            js = slice(512 * jh, 512 * jh + 512)
                            nc.tensor.matmul(v_ps[:, js], xv[:, i, :],
                                             wv_sb[:, i, js], start=(i == 0),
                                             stop=(i == EI - 1))
                    vu = pares.tile([128, E], bf16, tag=f"vu{t % 8}",
                                    name=f"vu{t}")
                    nc.vector.scalar_tensor_tensor(
                        out=vu, in0=v_ps, scalar=1.0, in1=bv_b.bitcast(f32),
                        op0=Alu.mult, op1=Alu.add)
                    xv_t[(t, "vu")] = vu

                    # rsqrt for each finished batch of 4; elu+kv staggered
                    # two tiles behind the projections to avoid PE stalls.
                    if t % 4 == 3:
                        rsqrt_batch(slice(t - 3, t + 1))
                    if t >= 2 and (t - 2) % 4 == 3:
                        for tb in range(t - 5, t - 1):
                            elu_k(tb)
                            kv_mms(tb)
                for tb in range(12, TT):
                    elu_k(tb)
                    kv_mms(tb)

                # pack the 16 good [64,64] blocks + ksum for the AllReduce
                pack = paw.tile([128, 520], f32, tag="pack")
                for pr in range(EI):
                    c = 64 * pr
                    nc.vector.tensor_copy(
                        out=pack[0:64, c:c + 64],
                        in_=kv_ps[0:64, 128 * pr:128 * pr + 64])
                    nc.vector.tensor_copy(
                        out=pack[64:128, c:c + 64],
                        in_=kv_ps[64:128, 128 * pr + 64:128 * pr + 128])
                nc.vector.tensor_copy(out=pack[:, 512:520], in_=ksum_ps)
                nc.sync.dma_start(out=cc_in, in_=pack)

            nc.gpsimd.collective_compute(
                "AllReduce", Alu.add, replica_groups=RG,
                ins=[cc_in[:, :]], outs=[cc_out[:, :]])

            # ============ Phase B1: q proj + LN stats (hides AllReduce) ====
            with tc.tile_pool(name="pu", bufs=1) as pu:
                u_t = [pu.tile([128, T], bf16, tag=f"u{j}", name=f"u{j}")
                       for j in range(EI)]
                rstd_row = pu.tile([1, T], f32, tag="rstd_row")
                rstd_b = pu.tile([128, T], bf16, tag="rstd_b")
                rstd_d = dp.tile([1, T], f32, tag="rstd_d")

                with tc.tile_pool(name="pb1", bufs=1) as pb1, \
                     tc.tile_pool(name="pbs", bufs=2) as pbs, \
                     tc.tile_pool(name="psb1", bufs=1, space="PSUM") as psb1:
                    xq = pb1.tile([128, EI, T], f32r, tag="xq")
                    wq_sb = pb1.tile([128, EI, E], f32r, tag="wq")
                    for i in range(EI):
                        nc.sync.dma_start(out=xq[:, i, :], in_=xqT_v[:, i, :])
                        nc.sync.dma_start(out=wq_sb[:, i, :],
                                          in_=wqT_v[:, i, :])
                    ssq_ps = psb1.tile([1, T], f32, tag="ssq")
                    ssq_pending = []
                    for j in range(EI):
                        for sh in range(2):
                            q_ps = psb1.tile([128, 1024], f32, tag="qps",
                                             bufs=2, name=f"qps{j}_{sh}")
                            for i in range(EI):
                                for sq in range(2):
                                    sl = slice(512 * sq, 512 * sq + 512)
                                    gl = slice(1024 * sh + 512 * sq,
                                               1024 * sh + 512 * sq + 512)
                                    nc.tensor.matmul(
                                        q_ps[:, sl],
                                        wq_sb[:, i, 128 * j:128 * j + 128],
                                        xq[:, i, gl],
                                        start=(i == 0), stop=(i == EI - 1))
                            # stagger ssq matmuls one block behind
                            if ssq_pending:
                                ssq_pending.pop(0)()
                            usq = pbs.tile([128, 1024], f32r, tag="usq",
                                           name=f"usq{j}_{sh}")
                            nc.scalar.activation(out=usq, in_=q_ps,
                                                 func=Act.Square,
                                                 bias=bq_sb[:, j:j + 1])
                            hs = slice(1024 * sh, 1024 * sh + 1024)
                            nc.scalar.activation(out=u_t[j][:, hs], in_=q_ps,
                                                 func=Act.Identity,
                                                 bias=bq_sb[:, j:j + 1])

                            def mk_ssq(j=j, sh=sh, usq=usq):
                                def emit():
                                    for sq in range(2):
                                        sl = slice(512 * sq, 512 * sq + 512)
                                        gl = slice(1024 * sh + 512 * sq,
                                                   1024 * sh + 512 * sq + 512)
                                        nc.tensor.matmul(
                                            ssq_ps[:, gl], onesC_sb,
                                            usq[:, sl], start=(j == 0),
                                            stop=(j == EI - 1),
                                            skip_group_check=True)
                                return emit
                            ssq_pending.append(mk_ssq())
                    for fn in ssq_pending:
                        fn()
                    # rstd = exp(-0.5*ln(ssq/E + eps)); broadcast to 128 p.
                    nc.scalar.activation(out=rstd_row, in_=ssq_ps,
                                         func=Act.Ln, scale=1.0 / E,
                                         bias=eps_sb[0:1, :])
                    nc.scalar.activation(out=rstd_row, in_=rstd_row,
                                         func=Act.Exp, scale=-0.5)
                    nc.sync.dma_start(out=rstd_d, in_=rstd_row)
                    nc.gpsimd.dma_start(out=rstd_b,
                                        in_=rstd_d.to_broadcast([128, T]))

                # ==== consume the AllReduce: kvbd (block-diag) + ksumB =====
                nc.sync.dma_start(out=ar_sb, in_=cc_out[:, :])
                nc.vector.memset(kvbd_bf, 0.0)
                nc.vector.memset(ksumB, 0.0)
                for pr in range(EI):
                    c = 64 * pr
                    nc.vector.tensor_copy(
                        out=kvbd_bf[0:64, 128 * pr:128 * pr + 64],
                        in_=ar_sb[0:64, c:c + 64])
                    nc.vector.tensor_copy(
                        out=kvbd_bf[64:128, 128 * pr + 64:128 * pr + 128],
                        in_=ar_sb[64:128, c:c + 64])
                # ksumB[:, j]: col-replicated per-head ksum. PE-transpose the
                # ksum columns to rows, then one matmul against a block-
                # diagonal ones pattern replicates each pair's ksum across
                # 64 columns; copy the head-matched halves out.
                with tc.tile_pool(name="psks", bufs=1,
                                  space="PSUM") as psks:
                    kst_ps = psks.tile([8, 128], f32, tag="kst")
                    nc.tensor.transpose(kst_ps, ar_sb[:, 512:520],
                                        ident_sb.bitcast(f32))
                    nc.vector.tensor_copy(out=ksumT, in_=kst_ps)
                    ksb_ps = psks.tile([128, 512], f32, tag="ksb")
                    nc.tensor.matmul(ksb_ps, ksumT, bones8,
                                     start=True, stop=True)
                    for pr in range(EI):
                        nc.vector.tensor_copy(
                            out=ksumB[0:64, pr, 0:64],
                            in_=ksb_ps[0:64, 64 * pr:64 * pr + 64])
                        nc.vector.tensor_copy(
                            out=ksumB[64:128, pr, 64:128],
                            in_=ksb_ps[64:128, 64 * pr:64 * pr + 64])

                # ============ Phase B2 + C: divide + num + out-proj ========
                with tc.tile_pool(name="pc1", bufs=1) as pc1, \
                     tc.tile_pool(name="pnum", bufs=1) as pnum, \
                     tc.tile_pool(name="pb2", bufs=3) as pb2, \
                     tc.tile_pool(name="pcl", bufs=2) as pcl, \
                     tc.tile_pool(name="psb2", bufs=2, space="PSUM") as psb2:
                    wo_sb = pc1.tile([128, EI, E], f32r, tag="wo")
                    for i in range(EI):
                        nc.sync.dma_start(out=wo_sb[:, i, :],
                                          in_=woT_v[:, i, :])
                    numT = [pnum.tile([128, T], f32r, tag=f"numT{j}",
                                      name=f"numT{j}") for j in range(EI)]

                    def elu_q(g, j):
                        gsl = slice(512 * g, 512 * g + 512)
                        t1 = pb2.tile([128, 512], f32, tag="t1")
                        nc.vector.tensor_tensor(out=t1, in0=u_t[j][:, gsl],
                                                in1=rstd_b[:, gsl],
                                                op=Alu.mult)
                        ee = pb2.tile([128, 512], bf16, tag="ee2")
                        nc.scalar.activation(out=ee, in_=t1, func=Act.Exp)
                        qf = pb2.tile([128, 512], bf16, tag="qf",
                                      name=f"qf{g}_{j}")
                        nc.scalar.activation(out=qf, in_=t1, func=Act.Relu)
                        nc.vector.scalar_tensor_tensor(
                            out=qf, in0=ee, scalar=1.0, in1=qf,
                            op0=Alu.min, op1=Alu.add)
                        return qf

                    def den_num(g, j, qf):
                        gsl = slice(512 * g, 512 * g + 512)
                        den_ps = psb2.tile([128, 512], f32, tag="dps")
                        nc.tensor.matmul(den_ps, ksumB[:, j, :], qf,
                                         start=True, stop=True)
                        invd = pb2.tile([128, 512], f32, tag="invd")
                        nc.vector.reciprocal_approx_fast(out=invd, in_=den_ps)
                        num_ps = psb2.tile([128, 512], f32, tag="nps")
                        nc.tensor.matmul(num_ps,
                                         kvbd_bf[:, 128 * j:128 * j + 128],
                                         qf, start=True, stop=True)
                        nc.vector.scalar_tensor_tensor(
                            out=numT[j][:, gsl], in0=num_ps, scalar=1.0,
                            in1=invd, op0=Alu.mult, op1=Alu.mult)

                    for g in range(4):
                        pend = []
                        for j in range(EI):
                            qf = elu_q(g, j)
                            pend.append((j, qf))
                            if len(pend) > 2:
                                jj, qq = pend.pop(0)
                                den_num(g, jj, qq)
                        for jj, qq in pend:
                            den_num(g, jj, qq)
                        for t4 in range(4):
                            tt = 4 * g + t4
                            tsl = slice(128 * tt, 128 * tt + 128)
                            o_ps = psb2.tile([128, E], f32, tag="ops")
                            for e in range(EI):
                                for jh in range(2):
                                    js = slice(512 * jh, 512 * jh + 512)
                                    nc.tensor.matmul(
                                        o_ps[:, js], numT[e][:, tsl],
                                        wo_sb[:, e, js], start=(e == 0),
                                        stop=(e == EI - 1))
                            o_sb = pcl.tile([128, E], f32, tag="osb")
                            nc.vector.scalar_tensor_tensor(
                                out=o_sb, in0=o_ps, scalar=1.0,
                                in1=bo_b.bitcast(f32), op0=Alu.mult,
                                op1=Alu.add)
                            nc.sync.dma_start(out=out_d[tsl, :], in_=o_sb)

    nc.finalize()
    return nc


def _prep_inputs(inputs):
    """Host-side fold + per-core shard maps."""
    f = np.float32
    Wq, bq = inputs["Wq"], inputs["bq"]
    Wk, bk = inputs["Wk"], inputs["bk"]
    Wv, bv = inputs["Wv"], inputs["bv"]
    Wo, bo = inputs["Wo"], inputs["bo"]
    for name in ("gq", "gk"):
        assert np.allclose(np.asarray(inputs[name]), 1.0), f"{name} != 1 unsupported"
    for name in ("betaq", "betak"):
        assert np.allclose(np.asarray(inputs[name]), 0.0), f"{name} != 0 unsupported"

    wqT = np.ascontiguousarray(np.asarray(Wq, f).T)
    wqT = wqT - wqT.mean(axis=1, keepdims=True)
    bqf = np.asarray(bq, f) - np.asarray(bq, f).mean()
    wkT = np.ascontiguousarray(np.asarray(Wk, f).T)
    wkT = wkT - wkT.mean(axis=1, keepdims=True)
    bkf = np.asarray(bk, f) - np.asarray(bk, f).mean()
    wvT = np.ascontiguousarray(np.asarray(Wv, f).T)
    woT = np.ascontiguousarray(np.asarray(Wo, f).T)

    shared = {
        "wqT": np.ascontiguousarray(wqT, f),
        "wkT": np.ascontiguousarray(wkT, f),
        "wvT": wvT,
        "woT": woT,
        "bq2d": np.ascontiguousarray(bqf.reshape(EI, 128).T, f),
        "bkR": np.ascontiguousarray(bkf.reshape(1, E), f),
        "bvR": np.ascontiguousarray(np.asarray(bv, f).reshape(1, E)),
        "boR": np.ascontiguousarray(np.asarray(bo, f).reshape(1, E)),
        "onesR": np.ones((1, 128), f),
        "onesC": np.ones((128, 1), f),
        "ident": np.eye(128, dtype=f),
        "bones8": np.kron(np.eye(8, dtype=f), np.ones((1, 64), f)),
        "zerosBD": np.zeros((128, E), f),
    }
    qe = np.asarray(inputs["query_embed"], f)
    ke = np.asarray(inputs["key_embed"], f)
    ve = np.asarray(inputs["value"], f)
    in_maps = []
    for c in range(NCORES):
        b, hh = divmod(c, 2)
        sl = slice(hh * T, (hh + 1) * T)
        m = dict(shared)
        m["xqT"] = np.ascontiguousarray(qe[b, sl, :].T)
        m["xkT"] = np.ascontiguousarray(ke[b, sl, :].T)
        m["xvT"] = np.ascontiguousarray(ve[b, sl, :].T)
        in_maps.append(m)
    return in_maps


def _run(inputs, trace=False):
    from concourse.bass_utils import run_bass_kernel_spmd

    if "nc" not in _NC_CACHE:
        _NC_CACHE["nc"] = _build_nc()
    nc = _NC_CACHE["nc"]
    in_maps = _prep_inputs(inputs)
    res = run_bass_kernel_spmd(nc, in_maps, core_ids=list(range(NCORES)),
                               trace=trace)
    out = np.empty((B, NSEQ, E), np.float32)
    for c in range(NCORES):
        b, hh = divmod(c, 2)
        out[b, hh * T:(hh + 1) * T, :] = res.results[c]["out"]
    return out, res


def kernel(**inputs):
    out, _ = _run(inputs, trace=False)
    return out


def kernel_traced(**inputs):
    """Like kernel() but also returns (exec_time_ns, trace_path)."""
    import sys, types
    try:
        import antenv
        if "antenv.axon_hooks" not in sys.modules:
            mod = types.ModuleType("antenv.axon_hooks")
            _h = [None]
            mod.set_axon_ntff_profile_hook = lambda h: _h.__setitem__(0, h)
            mod.get_axon_ntff_profile_hook = lambda: _h[0]
            sys.modules["antenv.axon_hooks"] = mod
            antenv.axon_hooks = mod
            from trn_agent_boot.trn_boot import _ntff_profile_via_ctypes
            mod.set_axon_ntff_profile_hook(
                _ntff_profile_via_ctypes("/opt/axon/libaxon_pjrt.so"))
    except Exception as e:  # profiling is best-effort
        print(f"NTFF hook setup failed: {e}")
    out, res = _run(inputs, trace=True)
    tp = res.instructions_and_trace[1] if res.instructions_and_trace else None
    return out, res.exec_time_ns, tp


# revision 39
# speedup vs baseline: 1.5563x; 1.1482x over previous
"""MultiHeadLinearAttention Trainium2 Bass kernel — 8-core SPMD.

Problem (per reference):
  q = elu(LN(Xq @ Wq.T + bq)) + 1 ; k = elu(LN(Xk @ Wk.T + bk)) + 1
  v = Xv @ Wv.T + bv
  kv = sum_n k[n] (x) v[n]   (per head, [D,D]);  ksum = sum_n k[n]
  out = ((q @ kv) / (q . ksum + 1e-8)) @ Wo.T + bo

Sharding: core c -> batch b = c//2, token half h = c%2 (2048 q AND k/v
tokens each). Per-pair (cores 2b, 2b+1) AllReduce of kv/ksum partials
(~266 KB) completes the sum over all 4096 k/v tokens of the batch.

Pipeline design (v2):
  Phase A (k/v): per-token-tile pipeline. rstd per tile via a DVE
    rsqrt bit-trick (no Ln/Exp -> no act-table switches), elu via
    min(exp(z),1)+relu(z) (exact), k_f/v in bf16 so the kv outer-
    product matmuls run 128-wide moving in bf16. kv/ksum accumulate
    in PSUM across all 16 tiles.
  Phase B1 (q proj) emitted between kv finish and AllReduce consume so
    the collective hides under q-projection matmuls.
  Phase B2+C: per-512-token-group pipeline. den is computed directly in
    broadcast form ([128,T]) via a replicated-ksum stationary, recip on
    DVE, divide fused into the num PSUM evacuation; out-projection for
    each group follows immediately so the PE never drains.

LayerNorm mean is folded into the weights on host (W~ = W^T(I-J/E),
b~ = b - mean(b)); gq/gk==1, betaq/betak==0 (asserted).
Projections run as float32r; attention-side matmuls run bf16.
"""

import os

import numpy as np

B, NSEQ, E, H, D = 4, 4096, 1024, 16, 64
NCORES = 8
T = NSEQ // 2          # tokens per core
TT = T // 128          # token tiles (16)
EI = E // 128          # feature tiles (8)
LN_EPS = 1e-5
RSQRT_MAGIC = 0x5F3759DF

_NC_CACHE = {}


def _build_nc(dbg=False):
    from concourse import bacc
    import concourse.bass as bass
    import concourse.mybir as mybir
    import concourse.tile as tile

    f32 = mybir.dt.float32
    f32r = mybir.dt.float32r
    bf16 = mybir.dt.bfloat16
    i32 = mybir.dt.int32
    Alu = mybir.AluOpType
    Act = mybir.ActivationFunctionType
    RG = [[0, 1], [2, 3], [4, 5], [6, 7]]

    nc = bacc.Bacc(num_devices=NCORES)

    xqT = nc.dram_tensor("xqT", [E, T], bf16, kind="ExternalInput")
    xkT = nc.dram_tensor("xkT", [E, T], bf16, kind="ExternalInput")
    xvT = nc.dram_tensor("xvT", [E, T], bf16, kind="ExternalInput")
    wqT = nc.dram_tensor("wqT", [E, E], bf16, kind="ExternalInput")
    wkT = nc.dram_tensor("wkT", [E, E], bf16, kind="ExternalInput")
    wvT = nc.dram_tensor("wvT", [E, E], bf16, kind="ExternalInput")
    woT = nc.dram_tensor("woT", [E, E], f32r, kind="ExternalInput")
    bq2d = nc.dram_tensor("bq2d", [128, EI], f32, kind="ExternalInput")
    bkR = nc.dram_tensor("bkR", [1, E], f32r, kind="ExternalInput")
    bvR = nc.dram_tensor("bvR", [1, E], f32r, kind="ExternalInput")
    boR = nc.dram_tensor("boR", [1, E], f32r, kind="ExternalInput")
    onesR = nc.dram_tensor("onesR", [1, 128], f32r, kind="ExternalInput")
    onesC = nc.dram_tensor("onesC", [128, 1], f32r, kind="ExternalInput")
    ident = nc.dram_tensor("ident", [128, 128], f32r, kind="ExternalInput")
    bones8_d = nc.dram_tensor("bones8", [8, 512], f32r, kind="ExternalInput")
    zerosBD = nc.dram_tensor("zerosBD", [128, E], f32r, kind="ExternalInput")
    out_d = nc.dram_tensor("out", [T, E], f32, kind="ExternalOutput")
    if dbg:
        d_ar = nc.dram_tensor("d_ar", [128, 520], f32, kind="ExternalOutput")
        d_ksb = nc.dram_tensor("d_ksb", [128, EI * 128], bf16,
                               kind="ExternalOutput")
        d_kvb = nc.dram_tensor("d_kvb", [128, E], bf16, kind="ExternalOutput")
        d_rst = nc.dram_tensor("d_rst", [128, T], bf16, kind="ExternalOutput")
        d_ssq = nc.dram_tensor("d_ssq", [1, T], f32, kind="ExternalOutput")
        d_inv = nc.dram_tensor("d_inv", [128, 512], f32, kind="ExternalOutput")
        d_qf = nc.dram_tensor("d_qf", [128, 512], bf16, kind="ExternalOutput")
        d_u = nc.dram_tensor("d_u", [128, T], bf16, kind="ExternalOutput")

    with tile.TileContext(nc) as tc:
        with tc.tile_pool(name="const", bufs=1) as cp, \
             tc.tile_pool(name="pbq", bufs=1) as pbq, \
             tc.tile_pool(name="dram", bufs=1, space="DRAM") as dp:
            onesR_sb = cp.tile([1, 128], f32r, tag="onesR_sb")
            nc.sync.dma_start(out=onesR_sb, in_=onesR[:, :])
            onesC_sb = cp.tile([128, 1], f32r, tag="onesC_sb")
            nc.sync.dma_start(out=onesC_sb, in_=onesC[:, :])
            zrow_sb = cp.tile([1, E], f32r, tag="zrow_sb")
            nc.sync.dma_start(out=zrow_sb, in_=zerosBD[0:1, :])
            ones_col_bf = cp.tile([128, 1], bf16, tag="ones_col_bf")
            nc.vector.memset(ones_col_bf, 1.0)
            eps_sb = cp.tile([128, 1], f32, tag="eps_sb")
            nc.vector.memset(eps_sb, LN_EPS)
            bq_sb = cp.tile([128, EI], f32, tag="bq_sb")
            nc.sync.dma_start(out=bq_sb, in_=bq2d[:, :])
            bo_b = cp.tile([128, E], f32r, tag="bo_b")
            nc.sync.dma_start(out=bo_b, in_=boR[:, :].to_broadcast([128, E]))
            kvbd_bf = cp.tile([128, E], bf16, tag="kvbd_bf")
            ksumB = cp.tile([128, EI, 128], bf16, tag="ksumB")
            ksumT = cp.tile([8, 128], f32r, tag="ksumT")
            ident_sb = cp.tile([128, 128], f32r, tag="ident_sb")
            nc.sync.dma_start(out=ident_sb, in_=ident[:, :])
            bones8 = cp.tile([8, 512], f32r, tag="bones8")
            nc.sync.dma_start(out=bones8, in_=bones8_d[:, :])
            ar_sb = cp.tile([128, 520], f32, tag="ar_sb")
            cc_in = dp.tile([128, 520], f32, tag="cc_in")
            cc_out = dp.tile([128, 520], f32, tag="cc_out")

            ss_all = cp.tile([128, TT], f32, tag="ss_all")
            rs_all = cp.tile([128, TT], f32, tag="rs_all")
            rsq_s1 = cp.tile([128, TT], f32, tag="rsq_s1")
            rsq_t = cp.tile([128, TT], f32, tag="rsq_t")

            xkT_v = xkT.rearrange("(i p) n -> p i n", p=128)
            xvT_v = xvT.rearrange("(i p) n -> p i n", p=128)
            xqT_v = xqT.rearrange("(i p) n -> p i n", p=128)
            wkT_v = wkT.rearrange("(i p) j -> p i j", p=128)
            wvT_v = wvT.rearrange("(i p) j -> p i j", p=128)
            wqT_v = wqT.rearrange("(i p) j -> p i j", p=128)
            woT_v = woT.rearrange("(i p) j -> p i j", p=128)

            def rsqrt_batch(sl):
                """rs_all[:, sl] = rsqrt(ss_all[:, sl]/E + eps) via the
                inverse-sqrt bit trick + 2 Newton iterations (DVE only)."""
                s1 = rsq_s1[:, sl]
                nc.vector.tensor_scalar(out=s1, in0=ss_all[:, sl],
                                        scalar1=1.0 / E, scalar2=LN_EPS,
                                        op0=Alu.mult, op1=Alu.add)
                y = rs_all[:, sl]
                yi = y.bitcast(i32)
                nc.vector.tensor_scalar(out=yi, in0=s1.bitcast(i32),
                                        scalar1=1, scalar2=None,
                                        op0=Alu.logical_shift_right)
                nc.vector.tensor_scalar(out=yi, in0=yi,
                                        scalar1=RSQRT_MAGIC, scalar2=-1,
                                        op0=Alu.subtract, op1=Alu.mult)
                t_ = rsq_t[:, sl]
                for _ in range(2):
                    nc.vector.tensor_tensor(out=t_, in0=y, in1=y, op=Alu.mult)
                    nc.vector.tensor_tensor(out=t_, in0=t_, in1=s1,
                                            op=Alu.mult)
                    nc.vector.tensor_scalar(out=t_, in0=t_, scalar1=-0.5,
                                            scalar2=1.5, op0=Alu.mult,
                                            op1=Alu.add)
                    nc.vector.tensor_tensor(out=y, in0=y, in1=t_, op=Alu.mult)

            # ============ Phase A: k/v proj + elu + kv (16-tile pipe) ======
            xq = pbq.tile([128, EI, T], bf16, tag="xq")
            wq_sb = pbq.tile([128, EI, E], bf16, tag="wq")

            with tc.tile_pool(name="paw", bufs=1) as paw, \
                 tc.tile_pool(name="pax", bufs=5) as pax, \
                 tc.tile_pool(name="pares", bufs=1) as pares, \
                 tc.tile_pool(name="pasc", bufs=3) as pasc, \
                 tc.tile_pool(name="pskv", bufs=1, space="PSUM") as pskv, \
                 tc.tile_pool(name="psproj", bufs=2, space="PSUM") as psproj:
                wk_sb = paw.tile([128, EI, E], bf16, tag="wk")
                wv_sb = paw.tile([128, EI, E], bf16, tag="wv")
                bk_b = paw.tile([128, E], f32r, tag="bk_b")
                nc.sync.dma_start(out=bk_b,
                                  in_=bkR[:, :].to_broadcast([128, E]))
                bv_b = paw.tile([128, E], f32r, tag="bv_b")
                nc.sync.dma_start(out=bv_b,
                                  in_=bvR[:, :].to_broadcast([128, E]))

                xk_t, xv_t = {}, {}

                def load_x(t):
                    tsl = slice(128 * t, 128 * t + 128)
                    xk = pax.tile([128, EI, 128], bf16, tag="xk",
                                  name=f"xk{t}")
                    nc.sync.dma_start(out=xk, in_=xkT_v[:, :, tsl])
                    xv = pax.tile([128, EI, 128], bf16, tag="xv",
                                  name=f"xv{t}")
                    nc.sync.dma_start(out=xv, in_=xvT_v[:, :, tsl])
                    xk_t[t], xv_t[t] = xk, xv

                # first x tiles before the bulk weight stream
                load_x(0)
                for i in range(EI):
                    nc.sync.dma_start(out=wk_sb[:, i, :], in_=wkT_v[:, i, :])
                    nc.sync.dma_start(out=wv_sb[:, i, :], in_=wvT_v[:, i, :])
                    if i < 3:
                        load_x(i + 1)

                kv_ps = pskv.tile([128, E], f32, tag="kv_ps")
                ksum_ps = pskv.tile([128, EI], f32, tag="ksum_ps")
                for h2 in range(2):
                    nc.tensor.matmul(kv_ps[:, 512 * h2:512 * h2 + 512],
                                     onesR_sb, zrow_sb[:, 0:512], start=True,
                                     stop=False, skip_group_check=True)
                nc.tensor.matmul(ksum_ps, onesR_sb, zrow_sb[:, 0:EI],
                                 start=True, stop=False, skip_group_check=True)

                ku_t, kf_t = {}, {}

                def elu_k(t):
                    rs = rs_all[:, t:t + 1]
                    kf = pares.tile([128, E], bf16, tag=f"kf{t % 3}",
                                    name=f"kf{t}")
                    nc.scalar.activation(out=kf, in_=ku_t[t], func=Act.Relu,
                                         scale=rs)
                    ee = pasc.tile([128, E], bf16, tag="ee")
                    nc.scalar.activation(out=ee, in_=ku_t[t], func=Act.Exp,
                                         scale=rs)
                    nc.vector.scalar_tensor_tensor(
                        out=kf, in0=ee, scalar=1.0, in1=kf,
                        op0=Alu.min, op1=Alu.add)
                    kf_t[t] = kf

                def kv_mms(t):
                    kf, vu = kf_t[t], xv_t[(t, "vu")]
                    for pr in range(EI):
                        kp = kf[:, 128 * pr:128 * pr + 128]
                        nc.tensor.matmul(
                            kv_ps[:, 128 * pr:128 * pr + 128], kp,
                            vu[:, 128 * pr:128 * pr + 128], start=False,
                            stop=(t == TT - 1), skip_group_check=True)
                        nc.tensor.matmul(
                            ksum_ps[:, pr:pr + 1], kp, ones_col_bf,
                            start=False, stop=(t == TT - 1),
                            skip_group_check=True)

                for t in range(TT):
                    if t >= 3 and t + 1 < TT:
                        load_x(t + 1)
                    if 4 <= t < 4 + EI:
                        i = t - 4
                        nc.sync.dma_start(out=xq[:, i, :], in_=xqT_v[:, i, :])
                        nc.sync.dma_start(out=wq_sb[:, i, :],
                                          in_=wqT_v[:, i, :])
                    xk, xv = xk_t[t], xv_t[t]
                    k_ps = psproj.tile([128, E], f32, tag="pps",
                                       name=f"kps{t}")
                    for i in range(EI):
                        for jh in range(2):
                            js = slice(512 * jh, 512 * jh + 512)
                            nc.tensor.matmul(k_ps[:, js], xk[:, i, :],
                                             wk_sb[:, i, js], start=(i == 0),
                                             stop=(i == EI - 1))
                    ku = pares.tile([128, E], f32, tag=f"ku{t % 8}",
                                    name=f"ku{t}")
                    nc.vector.scalar_tensor_tensor(
                        out=ku, in0=k_ps, scalar=1.0, in1=bk_b.bitcast(f32),
                        op0=Alu.mult, op1=Alu.add)
                    scrap = pasc.tile([128, E], bf16, tag="scrap")
                    nc.scalar.activation(out=scrap, in_=ku, func=Act.Square,
                                         accum_out=ss_all[:, t:t + 1])
                    ku_t[t] = ku

                    v_ps = psproj.tile([128, E], f32, tag="pps",
                                       name=f"vps{t}")
                    for i in range(EI):
                        for jh in range(2):
                            js = slice(512 * jh, 512 * jh + 512)
                            nc.tensor.matmul(v_ps[:, js], xv[:, i, :],
                                             wv_sb[:, i, js], start=(i == 0),
                                             stop=(i == EI - 1))
                    vu = pares.tile([128, E], bf16, tag=f"vu{t % 8}",
                                    name=f"vu{t}")
                    nc.vector.scalar_tensor_tensor(
                        out=vu, in0=v_ps, scalar=1.0, in1=bv_b.bitcast(f32),
                        op0=Alu.mult, op1=Alu.add)
                    xv_t[(t, "vu")] = vu

                    # rsqrt for each finished batch of 4; elu+kv staggered
                    # two tiles behind the projections to avoid PE stalls.
                    if t % 4 == 3:
                        rsqrt_batch(slice(t - 3, t + 1))
                    if t >= 2 and (t - 2) % 4 == 3:
                        for tb in range(t - 5, t - 1):
                            elu_k(tb)
                            kv_mms(tb)
                for tb in range(12, TT):
                    elu_k(tb)
                    kv_mms(tb)

                # pack the 16 good [64,64] blocks + ksum for the AllReduce
                pack = paw.tile([128, 520], f32, tag="pack")
                for pr in range(EI):
                    c = 64 * pr
                    nc.vector.tensor_copy(
                        out=pack[0:64, c:c + 64],
                        in_=kv_ps[0:64, 128 * pr:128 * pr + 64])
                    nc.vector.tensor_copy(
                        out=pack[64:128, c:c + 64],
                        in_=kv_ps[64:128, 128 * pr + 64:128 * pr + 128])
                nc.vector.tensor_copy(out=pack[:, 512:520], in_=ksum_ps)
                nc.sync.dma_start(out=cc_in, in_=pack)

            nc.gpsimd.collective_compute(
                "AllReduce", Alu.add, replica_groups=RG,
                ins=[cc_in[:, :]], outs=[cc_out[:, :]])

            # ============ Phase B1: q proj + LN stats (hides AllReduce) ====
            # consume the AllReduce on the idle SWDGE queue
            nc.gpsimd.dma_start(out=ar_sb, in_=cc_out[:, :])
            nc.vector.memset(kvbd_bf, 0.0)
            nc.vector.memset(ksumB, 0.0)
            for pr in range(EI):
                c = 64 * pr
                nc.vector.tensor_copy(
                    out=kvbd_bf[0:64, 128 * pr:128 * pr + 64],
                    in_=ar_sb[0:64, c:c + 64])
                nc.vector.tensor_copy(
                    out=kvbd_bf[64:128, 128 * pr + 64:128 * pr + 128],
                    in_=ar_sb[64:128, c:c + 64])

            with tc.tile_pool(name="pu", bufs=1) as pu:
                u_t = [pu.tile([128, T], bf16, tag=f"u{j}", name=f"u{j}")
                       for j in range(EI)]
                rstd_row = pu.tile([1, T], f32r, tag="rstd_row")
                rstd_b = pu.tile([128, T], bf16, tag="rstd_b")
                ssq_sb = pu.tile([1, T], f32, tag="ssq_sb")
                nc.vector.memset(ssq_sb, 0.0)

                with tc.tile_pool(name="pbs", bufs=2) as pbs, \
                     tc.tile_pool(name="psb1", bufs=1, space="PSUM") as psb1:
                    ssq_pending = []

                    def rstd_half(h):
                        hsl = slice(1024 * h, 1024 * h + 1024)
                        nc.scalar.activation(out=rstd_row[0:1, hsl],
                                             in_=ssq_sb[0:1, hsl],
                                             func=Act.Ln,
                                             scale=1.0 / E,
                                             bias=eps_sb[0:1, :])
                        nc.scalar.activation(out=rstd_row[0:1, hsl],
                                             in_=rstd_row[0:1, hsl],
                                             func=Act.Exp, scale=-0.5)
                        for cc in range(2):
                            csl = slice(1024 * h + 512 * cc,
                                        1024 * h + 512 * cc + 512)
                            r_ps = psb1.tile([128, 512], f32, tag="rps",
                                             bufs=2, name=f"rps{h}_{cc}")
                            nc.tensor.matmul(r_ps, onesR_sb,
                                             rstd_row[0:1, csl], start=True,
                                             stop=True)
                            nc.scalar.activation(out=rstd_b[:, csl],
                                                 in_=r_ps, func=Act.Copy)

                    def ksumb_build():
                        # ksumB[:, j]: col-replicated per-head ksum via
                        # PE-transpose + block-diagonal ones matmul.
                        kst_ps = psb1.tile([8, 128], f32, tag="rps",
                                           bufs=2, name="kst")
                        nc.tensor.transpose(kst_ps, ar_sb[:, 512:520],
                                            ident_sb.bitcast(f32))
                        nc.vector.tensor_copy(out=ksumT, in_=kst_ps)
                        ksb_ps = psb1.tile([128, 512], f32, tag="rps",
                                           bufs=2, name="ksb")
                        nc.tensor.matmul(ksb_ps, ksumT, bones8,
                                         start=True, stop=True)
                        for pr in range(EI):
                            nc.vector.tensor_copy(
                                out=ksumB[0:64, pr, 0:64],
                                in_=ksb_ps[0:64, 64 * pr:64 * pr + 64])
                            nc.vector.tensor_copy(
                                out=ksumB[64:128, pr, 64:128],
                                in_=ksb_ps[64:128, 64 * pr:64 * pr + 64])

                    for sh in range(2):
                        for j in range(EI):
                            q_ps = psb1.tile([128, 1024], f32, tag="qps",
                                             bufs=2, name=f"qps{sh}_{j}")
                            for i in range(EI):
                                for sq in range(2):
                                    sl = slice(512 * sq, 512 * sq + 512)
                                    gl = slice(1024 * sh + 512 * sq,
                                               1024 * sh + 512 * sq + 512)
                                    nc.tensor.matmul(
                                        q_ps[:, sl],
                                        wq_sb[:, i, 128 * j:128 * j + 128],
                                        xq[:, i, gl],
                                        start=(i == 0), stop=(i == EI - 1))
                            # stagger ssq matmuls one block behind
                            if ssq_pending:
                                ssq_pending.pop(0)()
                            usq = pbs.tile([128, 1024], f32r, tag="usq",
                                           name=f"usq{sh}_{j}")
                            nc.scalar.activation(out=usq, in_=q_ps,
                                                 func=Act.Square,
                                                 bias=bq_sb[:, j:j + 1])
                            hs = slice(1024 * sh, 1024 * sh + 1024)
                            nc.scalar.activation(out=u_t[j][:, hs], in_=q_ps,
                                                 func=Act.Identity,
                                                 bias=bq_sb[:, j:j + 1])

                            def mk_ssq(j=j, sh=sh, usq=usq):
                                def emit():
                                    for sq in range(2):
                                        sl = slice(512 * sq, 512 * sq + 512)
                                        gl = slice(1024 * sh + 512 * sq,
                                                   1024 * sh + 512 * sq + 512)
                                        sp = psb1.tile([1, 512], f32,
                                                       tag="ssqp", bufs=2)
                                        nc.tensor.matmul(
                                            sp, onesC_sb, usq[:, sl],
                                            start=True, stop=True)
                                        nc.vector.tensor_tensor(
                                            out=ssq_sb[0:1, gl],
                                            in0=ssq_sb[0:1, gl], in1=sp,
                                            op=Alu.add)
                                return emit
                            ssq_pending.append(mk_ssq())
                            if sh == 1 and j == 1:
                                rstd_half(0)
                            if sh == 1 and j == 4:
                                ksumb_build()
                    for fn in ssq_pending:
                        fn()
                    rstd_half(1)
                    if dbg:
                        nc.sync.dma_start(out=d_ar[:, :], in_=ar_sb)
                        nc.sync.dma_start(
                            out=d_ksb[:, :],
                            in_=ksumB.rearrange("p a b -> p (a b)"))
                        nc.sync.dma_start(out=d_kvb[:, :], in_=kvbd_bf)
                        nc.sync.dma_start(out=d_rst[:, :], in_=rstd_b)
                        nc.sync.dma_start(out=d_ssq[:, :], in_=ssq_sb)
                        nc.sync.dma_start(out=d_u[:, :], in_=u_t[0])

                # ============ Phase B2 + C: divide + num + out-proj ========
                with tc.tile_pool(name="pc1", bufs=1) as pc1, \
                     tc.tile_pool(name="pb2", bufs=3) as pb2, \
                     tc.tile_pool(name="pcl", bufs=2) as pcl, \
                     tc.tile_pool(name="psb2", bufs=2, space="PSUM") as psb2:
                    wo_sb = pc1.tile([128, EI, E], f32r, tag="wo")
                    for i in range(EI):
                        nc.sync.dma_start(out=wo_sb[:, i, :],
                                          in_=woT_v[:, i, :])
                    numT = {}

                    def elu_q(g, j):
                        gsl = slice(512 * g, 512 * g + 512)
                        t1 = pb2.tile([128, 512], f32, tag="t1")
                        nc.vector.tensor_tensor(out=t1, in0=u_t[j][:, gsl],
                                                in1=rstd_b[:, gsl],
                                                op=Alu.mult)
                        ee = pb2.tile([128, 512], bf16, tag="ee2")
                        nc.scalar.activation(out=ee, in_=t1, func=Act.Exp)
                        qf = pb2.tile([128, 512], bf16, tag="qf",
                                      name=f"qf{g}_{j}")
                        nc.scalar.activation(out=qf, in_=t1, func=Act.Relu)
                        nc.vector.scalar_tensor_tensor(
                            out=qf, in0=ee, scalar=1.0, in1=qf,
                            op0=Alu.min, op1=Alu.add)
                        return qf

                    def den_num(g, j, qf):
                        gsl = slice(512 * g, 512 * g + 512)
                        den_ps = psb2.tile([128, 512], f32, tag="dps")
                        nc.tensor.matmul(den_ps, ksumB[:, j, :], qf,
                                         start=True, stop=True)
                        invd = pb2.tile([128, 512], f32, tag="invd", bufs=2)
                        nc.vector.reciprocal_approx_fast(out=invd, in_=den_ps)
                        num_ps = psb2.tile([128, 512], f32, tag="nps")
                        nc.tensor.matmul(num_ps,
                                         kvbd_bf[:, 128 * j:128 * j + 128],
                                         qf, start=True, stop=True)
                        nt = pb2.tile([128, 512], f32r, tag=f"numT{j}",
                                      bufs=2, name=f"numT{g}_{j}")
                        numT[j] = nt
                        nc.vector.scalar_tensor_tensor(
                            out=nt, in0=num_ps, scalar=1.0,
                            in1=invd, op0=Alu.mult, op1=Alu.mult)
                        if dbg and g == 0 and j == 0:
                            nc.sync.dma_start(out=d_inv[:, :], in_=invd)
                            nc.sync.dma_start(out=d_qf[:, :], in_=qf)

                    for g in range(4):
                        pend = []
                        for j in range(EI):
                            qf = elu_q(g, j)
                            pend.append((j, qf))
                            if len(pend) > 2:
                                jj, qq = pend.pop(0)
                                den_num(g, jj, qq)
                        for jj, qq in pend:
                            den_num(g, jj, qq)
                        for t4 in range(4):
                            tt = 4 * g + t4
                            tsl = slice(128 * tt, 128 * tt + 128)
                            t4sl = slice(128 * t4, 128 * t4 + 128)
                            o_ps = psb2.tile([128, E], f32, tag="ops")
                            for e in range(EI):
                                for jh in range(2):
                                    js = slice(512 * jh, 512 * jh + 512)
                                    nc.tensor.matmul(
                                        o_ps[:, js], numT[e][:, t4sl],
                                        wo_sb[:, e, js], start=(e == 0),
                                        stop=(e == EI - 1))
                            o_sb = pcl.tile([128, E], f32, tag="osb")
                            nc.vector.scalar_tensor_tensor(
                                out=o_sb, in0=o_ps, scalar=1.0,
                                in1=bo_b.bitcast(f32), op0=Alu.mult,
                                op1=Alu.add)
                            nc.sync.dma_start(out=out_d[tsl, :], in_=o_sb)

    nc.finalize()
    return nc


def _prep_inputs(inputs):
    """Host-side fold + per-core shard maps."""
    import ml_dtypes
    f = np.float32
    bf = ml_dtypes.bfloat16
    Wq, bq = inputs["Wq"], inputs["bq"]
    Wk, bk = inputs["Wk"], inputs["bk"]
    Wv, bv = inputs["Wv"], inputs["bv"]
    Wo, bo = inputs["Wo"], inputs["bo"]
    for name in ("gq", "gk"):
        assert np.allclose(np.asarray(inputs[name]), 1.0), f"{name} != 1 unsupported"
    for name in ("betaq", "betak"):
        assert np.allclose(np.asarray(inputs[name]), 0.0), f"{name} != 0 unsupported"

    wqT = np.ascontiguousarray(np.asarray(Wq, f).T)
    wqT = wqT - wqT.mean(axis=1, keepdims=True)
    bqf = np.asarray(bq, f) - np.asarray(bq, f).mean()
    wkT = np.ascontiguousarray(np.asarray(Wk, f).T)
    wkT = wkT - wkT.mean(axis=1, keepdims=True)
    bkf = np.asarray(bk, f) - np.asarray(bk, f).mean()
    wvT = np.ascontiguousarray(np.asarray(Wv, f).T)
    woT = np.ascontiguousarray(np.asarray(Wo, f).T)

    shared = {
        "wqT": np.ascontiguousarray(wqT).astype(bf),
        "wkT": np.ascontiguousarray(wkT).astype(bf),
        "wvT": wvT.astype(bf),
        "woT": woT,
        "bq2d": np.ascontiguousarray(bqf.reshape(EI, 128).T, f),
        "bkR": np.ascontiguousarray(bkf.reshape(1, E), f),
        "bvR": np.ascontiguousarray(np.asarray(bv, f).reshape(1, E)),
        "boR": np.ascontiguousarray(np.asarray(bo, f).reshape(1, E)),
        "onesR": np.ones((1, 128), f),
        "onesC": np.ones((128, 1), f),
        "ident": np.eye(128, dtype=f),
        "bones8": np.kron(np.eye(8, dtype=f), np.ones((1, 64), f)),
        "zerosBD": np.zeros((128, E), f),
    }
    qe = np.asarray(inputs["query_embed"], f)
    ke = np.asarray(inputs["key_embed"], f)
    ve = np.asarray(inputs["value"], f)
    in_maps = []
    for c in range(NCORES):
        b, hh = divmod(c, 2)
        sl = slice(hh * T, (hh + 1) * T)
        m = dict(shared)
        m["xqT"] = np.ascontiguousarray(qe[b, sl, :].T).astype(bf)
        m["xkT"] = np.ascontiguousarray(ke[b, sl, :].T).astype(bf)
        m["xvT"] = np.ascontiguousarray(ve[b, sl, :].T).astype(bf)
        in_maps.append(m)
    return in_maps


def _run(inputs, trace=False):
    from concourse.bass_utils import run_bass_kernel_spmd

    import os as _os
    dbg = bool(int(_os.environ.get("KERNEL_DBG", "0")))
    key = "nc_dbg" if dbg else "nc"
    if key not in _NC_CACHE:
        _NC_CACHE[key] = _build_nc(dbg=dbg)
    nc = _NC_CACHE[key]
    in_maps = _prep_inputs(inputs)
    res = run_bass_kernel_spmd(nc, in_maps, core_ids=list(range(NCORES)),
                               trace=trace)
    out = np.empty((B, NSEQ, E), np.float32)
    for c in range(NCORES):
        b, hh = divmod(c, 2)
        out[b, hh * T:(hh + 1) * T, :] = res.results[c]["out"]
    return out, res


def kernel(**inputs):
    out, _ = _run(inputs, trace=False)
    return out


def kernel_traced(**inputs):
    """Like kernel() but also returns (exec_time_ns, trace_path)."""
    import sys, types
    try:
        import antenv
        if "antenv.axon_hooks" not in sys.modules:
            mod = types.ModuleType("antenv.axon_hooks")
            _h = [None]
            mod.set_axon_ntff_profile_hook = lambda h: _h.__setitem__(0, h)
            mod.get_axon_ntff_profile_hook = lambda: _h[0]
            sys.modules["antenv.axon_hooks"] = mod
            antenv.axon_hooks = mod
            from trn_agent_boot.trn_boot import _ntff_profile_via_ctypes
            mod.set_axon_ntff_profile_hook(
                _ntff_profile_via_ctypes("/opt/axon/libaxon_pjrt.so"))
    except Exception as e:  # profiling is best-effort
        print(f"NTFF hook setup failed: {e}")
    out, res = _run(inputs, trace=True)
    tp = res.instructions_and_trace[1] if res.instructions_and_trace else None
    return out, res.exec_time_ns, tp


# revision 43
# speedup vs baseline: 1.6296x; 1.0471x over previous
"""MultiHeadLinearAttention Trainium2 Bass kernel — 8-core SPMD.

Problem (per reference):
  q = elu(LN(Xq @ Wq.T + bq)) + 1 ; k = elu(LN(Xk @ Wk.T + bk)) + 1
  v = Xv @ Wv.T + bv
  kv = sum_n k[n] (x) v[n]   (per head, [D,D]);  ksum = sum_n k[n]
  out = ((q @ kv) / (q . ksum + 1e-8)) @ Wo.T + bo

Sharding: core c -> batch b = c//2, token half h = c%2 (2048 q AND k/v
tokens each). Per-pair (cores 2b, 2b+1) AllReduce of kv/ksum partials
(~266 KB) completes the sum over all 4096 k/v tokens of the batch.

Pipeline design (v2):
  Phase A (k/v): per-token-tile pipeline. rstd per tile via a DVE
    rsqrt bit-trick (no Ln/Exp -> no act-table switches), elu via
    min(exp(z),1)+relu(z) (exact), k_f/v in bf16 so the kv outer-
    product matmuls run 128-wide moving in bf16. kv/ksum accumulate
    in PSUM across all 16 tiles.
  Phase B1 (q proj) emitted between kv finish and AllReduce consume so
    the collective hides under q-projection matmuls.
  Phase B2+C: per-512-token-group pipeline. den is computed directly in
    broadcast form ([128,T]) via a replicated-ksum stationary, recip on
    DVE, divide fused into the num PSUM evacuation; out-projection for
    each group follows immediately so the PE never drains.

LayerNorm mean is folded into the weights on host (W~ = W^T(I-J/E),
b~ = b - mean(b)); gq/gk==1, betaq/betak==0 (asserted).
Projections run as float32r; attention-side matmuls run bf16.
"""

import os

import numpy as np

B, NSEQ, E, H, D = 4, 4096, 1024, 16, 64
NCORES = 8
T = NSEQ // 2          # tokens per core
TT = T // 128          # token tiles (16)
EI = E // 128          # feature tiles (8)
LN_EPS = 1e-5
RSQRT_MAGIC = 0x5F3759DF

_NC_CACHE = {}


def _build_nc(dbg=False):
    from concourse import bacc
    import concourse.bass as bass
    import concourse.mybir as mybir
    import concourse.tile as tile

    f32 = mybir.dt.float32
    f32r = mybir.dt.float32r
    bf16 = mybir.dt.bfloat16
    i32 = mybir.dt.int32
    Alu = mybir.AluOpType
    Act = mybir.ActivationFunctionType
    RG = [[0, 1], [2, 3], [4, 5], [6, 7]]

    nc = bacc.Bacc(num_devices=NCORES)

    xqT = nc.dram_tensor("xqT", [E, T], bf16, kind="ExternalInput")
    xkT = nc.dram_tensor("xkT", [E, T], bf16, kind="ExternalInput")
    xvT = nc.dram_tensor("xvT", [E, T], bf16, kind="ExternalInput")
    wqT = nc.dram_tensor("wqT", [E, E], bf16, kind="ExternalInput")
    wkT = nc.dram_tensor("wkT", [E, E], bf16, kind="ExternalInput")
    wvT = nc.dram_tensor("wvT", [E, E], bf16, kind="ExternalInput")
    woT = nc.dram_tensor("woT", [E, E], f32r, kind="ExternalInput")
    bq2d = nc.dram_tensor("bq2d", [128, EI], f32, kind="ExternalInput")
    bkR = nc.dram_tensor("bkR", [1, E], f32r, kind="ExternalInput")
    bvR = nc.dram_tensor("bvR", [1, E], f32r, kind="ExternalInput")
    boR = nc.dram_tensor("boR", [1, E], f32r, kind="ExternalInput")
    onesR = nc.dram_tensor("onesR", [1, 128], f32r, kind="ExternalInput")
    onesC = nc.dram_tensor("onesC", [128, 1], f32r, kind="ExternalInput")
    ident = nc.dram_tensor("ident", [128, 128], f32r, kind="ExternalInput")
    bones8_d = nc.dram_tensor("bones8", [8, 512], f32r, kind="ExternalInput")
    zerosBD = nc.dram_tensor("zerosBD", [128, E], f32r, kind="ExternalInput")
    out_d = nc.dram_tensor("out", [T, E], f32, kind="ExternalOutput")
    if dbg:
        d_ar = nc.dram_tensor("d_ar", [128, 520], f32, kind="ExternalOutput")
        d_ksb = nc.dram_tensor("d_ksb", [128, EI * 128], bf16,
                               kind="ExternalOutput")
        d_kvb = nc.dram_tensor("d_kvb", [128, E], bf16, kind="ExternalOutput")
        d_rst = nc.dram_tensor("d_rst", [128, T], bf16, kind="ExternalOutput")
        d_ssq = nc.dram_tensor("d_ssq", [1, T], f32, kind="ExternalOutput")
        d_inv = nc.dram_tensor("d_inv", [128, 512], f32, kind="ExternalOutput")
        d_qf = nc.dram_tensor("d_qf", [128, 512], bf16, kind="ExternalOutput")
        d_u = nc.dram_tensor("d_u", [128, T], bf16, kind="ExternalOutput")

    with tile.TileContext(nc) as tc:
        with tc.tile_pool(name="const", bufs=1) as cp, \
             tc.tile_pool(name="pbq", bufs=1) as pbq, \
             tc.tile_pool(name="dram", bufs=1, space="DRAM") as dp:
            onesR_sb = cp.tile([1, 128], f32r, tag="onesR_sb")
            nc.sync.dma_start(out=onesR_sb, in_=onesR[:, :])
            onesC_sb = cp.tile([128, 1], f32r, tag="onesC_sb")
            nc.sync.dma_start(out=onesC_sb, in_=onesC[:, :])
            zrow_sb = cp.tile([1, E], f32r, tag="zrow_sb")
            nc.sync.dma_start(out=zrow_sb, in_=zerosBD[0:1, :])
            ones_col_bf = cp.tile([128, 1], bf16, tag="ones_col_bf")
            nc.vector.memset(ones_col_bf, 1.0)
            eps_sb = cp.tile([128, 1], f32, tag="eps_sb")
            nc.vector.memset(eps_sb, LN_EPS)
            bq_sb = cp.tile([128, EI], f32, tag="bq_sb")
            nc.sync.dma_start(out=bq_sb, in_=bq2d[:, :])
            bo_b = cp.tile([128, E], f32r, tag="bo_b")
            nc.sync.dma_start(out=bo_b, in_=boR[:, :].to_broadcast([128, E]))
            kvbd_bf = cp.tile([128, E], bf16, tag="kvbd_bf")
            ksumB = cp.tile([128, EI, 128], bf16, tag="ksumB")
            ksumT = cp.tile([8, 128], f32r, tag="ksumT")
            ident_sb = cp.tile([128, 128], f32r, tag="ident_sb")
            nc.sync.dma_start(out=ident_sb, in_=ident[:, :])
            bones8 = cp.tile([8, 512], f32r, tag="bones8")
            nc.sync.dma_start(out=bones8, in_=bones8_d[:, :])
            ar_sb = cp.tile([128, 520], f32, tag="ar_sb")
            cc_in = dp.tile([128, 520], f32, tag="cc_in")
            cc_out = dp.tile([128, 520], f32, tag="cc_out")

            ss_all = cp.tile([128, TT], f32, tag="ss_all")
            rs_all = cp.tile([128, TT], f32, tag="rs_all")
            rsq_s1 = cp.tile([128, TT], f32, tag="rsq_s1")
            rsq_t = cp.tile([128, TT], f32, tag="rsq_t")

            xkT_v = xkT.rearrange("(i p) n -> p i n", p=128)
            xvT_v = xvT.rearrange("(i p) n -> p i n", p=128)
            xqT_v = xqT.rearrange("(i p) n -> p i n", p=128)
            wkT_v = wkT.rearrange("(i p) j -> p i j", p=128)
            wvT_v = wvT.rearrange("(i p) j -> p i j", p=128)
            wqT_v = wqT.rearrange("(i p) j -> p i j", p=128)
            woT_v = woT.rearrange("(i p) j -> p i j", p=128)

            def rsqrt_batch(sl):
                """rs_all[:, sl] = rsqrt(ss_all[:, sl]/E + eps) via the
                inverse-sqrt bit trick + 2 Newton iterations (DVE only)."""
                s1 = rsq_s1[:, sl]
                nc.vector.tensor_scalar(out=s1, in0=ss_all[:, sl],
                                        scalar1=1.0 / E, scalar2=LN_EPS,
                                        op0=Alu.mult, op1=Alu.add)
                y = rs_all[:, sl]
                yi = y.bitcast(i32)
                nc.vector.tensor_scalar(out=yi, in0=s1.bitcast(i32),
                                        scalar1=1, scalar2=None,
                                        op0=Alu.logical_shift_right)
                nc.vector.tensor_scalar(out=yi, in0=yi,
                                        scalar1=RSQRT_MAGIC, scalar2=-1,
                                        op0=Alu.subtract, op1=Alu.mult)
                t_ = rsq_t[:, sl]
                for _ in range(2):
                    nc.vector.tensor_tensor(out=t_, in0=y, in1=y, op=Alu.mult)
                    nc.vector.tensor_tensor(out=t_, in0=t_, in1=s1,
                                            op=Alu.mult)
                    nc.vector.tensor_scalar(out=t_, in0=t_, scalar1=-0.5,
                                            scalar2=1.5, op0=Alu.mult,
                                            op1=Alu.add)
                    nc.vector.tensor_tensor(out=y, in0=y, in1=t_, op=Alu.mult)

            # ============ Phase A: k/v proj + elu + kv (16-tile pipe) ======
            xq = pbq.tile([128, EI, T], bf16, tag="xq")
            wq_sb = pbq.tile([128, EI, E], bf16, tag="wq")

            with tc.tile_pool(name="paw", bufs=1) as paw, \
                 tc.tile_pool(name="pax", bufs=5) as pax, \
                 tc.tile_pool(name="pares", bufs=1) as pares, \
                 tc.tile_pool(name="pasc", bufs=3) as pasc, \
                 tc.tile_pool(name="pskv", bufs=1, space="PSUM") as pskv, \
                 tc.tile_pool(name="psproj", bufs=2, space="PSUM") as psproj:
                wk_sb = paw.tile([128, EI, E], bf16, tag="wk")
                wv_sb = paw.tile([128, EI, E], bf16, tag="wv")
                bk_b = paw.tile([128, E], f32r, tag="bk_b")
                nc.sync.dma_start(out=bk_b,
                                  in_=bkR[:, :].to_broadcast([128, E]))
                bv_b = paw.tile([128, E], f32r, tag="bv_b")
                nc.sync.dma_start(out=bv_b,
                                  in_=bvR[:, :].to_broadcast([128, E]))

                xk_t, xv_t = {}, {}

                def load_x(t):
                    tsl = slice(128 * t, 128 * t + 128)
                    xk = pax.tile([128, EI, 128], bf16, tag="xk",
                                  name=f"xk{t}")
                    nc.sync.dma_start(out=xk, in_=xkT_v[:, :, tsl])
                    xv = pax.tile([128, EI, 128], bf16, tag="xv",
                                  name=f"xv{t}")
                    nc.sync.dma_start(out=xv, in_=xvT_v[:, :, tsl])
                    xk_t[t], xv_t[t] = xk, xv

                # first x tile, then all wk (k-proj gates on full wk),
                # then wv interleaved with the next x tiles
                load_x(0)
                for i in range(EI):
                    nc.sync.dma_start(out=wk_sb[:, i, :], in_=wkT_v[:, i, :])
                for i in range(EI):
                    nc.sync.dma_start(out=wv_sb[:, i, :], in_=wvT_v[:, i, :])
                    if i < 3:
                        load_x(i + 1)

                kv_ps = pskv.tile([128, E], f32, tag="kv_ps")
                ksum_ps = pskv.tile([128, EI], f32, tag="ksum_ps")
                for h2 in range(2):
                    nc.tensor.matmul(kv_ps[:, 512 * h2:512 * h2 + 512],
                                     onesR_sb, zrow_sb[:, 0:512], start=True,
                                     stop=False, skip_group_check=True)
                nc.tensor.matmul(ksum_ps, onesR_sb, zrow_sb[:, 0:EI],
                                 start=True, stop=False, skip_group_check=True)

                ku_t, kf_t = {}, {}

                def elu_k(t):
                    rs = rs_all[:, t:t + 1]
                    kf = pares.tile([128, E], bf16, tag=f"kf{t % 3}",
                                    name=f"kf{t}")
                    nc.scalar.activation(out=kf, in_=ku_t[t], func=Act.Relu,
                                         scale=rs)
                    ee = pasc.tile([128, E], bf16, tag="ee")
                    nc.scalar.activation(out=ee, in_=ku_t[t], func=Act.Exp,
                                         scale=rs)
                    nc.vector.scalar_tensor_tensor(
                        out=kf, in0=ee, scalar=1.0, in1=kf,
                        op0=Alu.min, op1=Alu.add)
                    kf_t[t] = kf

                def kv_mms(t):
                    kf, vu = kf_t[t], xv_t[(t, "vu")]
                    for pr in range(EI):
                        kp = kf[:, 128 * pr:128 * pr + 128]
                        nc.tensor.matmul(
                            kv_ps[:, 128 * pr:128 * pr + 128], kp,
                            vu[:, 128 * pr:128 * pr + 128], start=False,
                            stop=(t == TT - 1), skip_group_check=True)
                        nc.tensor.matmul(
                            ksum_ps[:, pr:pr + 1], kp, ones_col_bf,
                            start=False, stop=(t == TT - 1),
                            skip_group_check=True)

                for t in range(TT):
                    if t >= 3 and t + 1 < TT:
                        load_x(t + 1)
                    if 4 <= t < 4 + EI:
                        i = t - 4
                        nc.sync.dma_start(out=xq[:, i, :], in_=xqT_v[:, i, :])
                        nc.sync.dma_start(out=wq_sb[:, i, :],
                                          in_=wqT_v[:, i, :])
                    xk, xv = xk_t[t], xv_t[t]
                    k_ps = psproj.tile([128, E], f32, tag="pps",
                                       name=f"kps{t}")
                    for i in range(EI):
                        for jh in range(2):
                            js = slice(512 * jh, 512 * jh + 512)
                            nc.tensor.matmul(k_ps[:, js], xk[:, i, :],
                                             wk_sb[:, i, js], start=(i == 0),
                                             stop=(i == EI - 1))
                    ku = pares.tile([128, E], f32, tag=f"ku{t % 8}",
                                    name=f"ku{t}")
                    nc.vector.scalar_tensor_tensor(
                        out=ku, in0=k_ps, scalar=1.0, in1=bk_b.bitcast(f32),
                        op0=Alu.mult, op1=Alu.add)
                    scrap = pasc.tile([128, E], bf16, tag="scrap")
                    nc.scalar.activation(out=scrap, in_=ku, func=Act.Square,
                                         accum_out=ss_all[:, t:t + 1])
                    ku_t[t] = ku

                    v_ps = psproj.tile([128, E], f32, tag="pps",
                                       name=f"vps{t}")
                    for i in range(EI):
                        for jh in range(2):
                            js = slice(512 * jh, 512 * jh + 512)
                            nc.tensor.matmul(v_ps[:, js], xv[:, i, :],
                                             wv_sb[:, i, js], start=(i == 0),
                                             stop=(i == EI - 1))
                    vu = pares.tile([128, E], bf16, tag=f"vu{t % 8}",
                                    name=f"vu{t}")
                    nc.vector.scalar_tensor_tensor(
                        out=vu, in0=v_ps, scalar=1.0, in1=bv_b.bitcast(f32),
                        op0=Alu.mult, op1=Alu.add)
                    xv_t[(t, "vu")] = vu

                    # rsqrt for each finished batch of 4; elu+kv staggered
                    # two tiles behind the projections to avoid PE stalls.
                    if t % 4 == 3:
                        rsqrt_batch(slice(t - 3, t + 1))
                    if t >= 2 and (t - 2) % 4 == 3:
                        for tb in range(t - 5, t - 1):
                            elu_k(tb)
                            kv_mms(tb)
                for tb in range(12, TT):
                    elu_k(tb)
                    kv_mms(tb)

                # pack the 16 good [64,64] blocks + ksum for the AllReduce
                pack = paw.tile([128, 520], f32, tag="pack")
                for pr in range(EI):
                    c = 64 * pr
                    nc.vector.tensor_copy(
                        out=pack[0:64, c:c + 64],
                        in_=kv_ps[0:64, 128 * pr:128 * pr + 64])
                    nc.vector.tensor_copy(
                        out=pack[64:128, c:c + 64],
                        in_=kv_ps[64:128, 128 * pr + 64:128 * pr + 128])
                nc.vector.tensor_copy(out=pack[:, 512:520], in_=ksum_ps)
                # jump the sync-queue backlog: SWDGE carries cc_in
                nc.gpsimd.dma_start(out=cc_in, in_=pack)

            nc.gpsimd.collective_compute(
                "AllReduce", Alu.add, replica_groups=RG,
                ins=[cc_in[:, :]], outs=[cc_out[:, :]])

            # ============ Phase B1: q proj + LN stats (hides AllReduce) ====
            # consume the AllReduce on the idle SWDGE queue
            nc.gpsimd.dma_start(out=ar_sb, in_=cc_out[:, :])
            nc.vector.memset(kvbd_bf, 0.0)
            nc.vector.memset(ksumB, 0.0)

            with tc.tile_pool(name="pu", bufs=1) as pu:
                u_t = [pu.tile([128, T], bf16, tag=f"u{j}", name=f"u{j}")
                       for j in range(EI)]
                rstd_row = pu.tile([1, T], f32r, tag="rstd_row")
                rstd_b = pu.tile([128, T], bf16, tag="rstd_b")
                ssq_sb = pu.tile([1, T], f32, tag="ssq_sb")
                nc.vector.memset(ssq_sb, 0.0)

                with tc.tile_pool(name="pbs", bufs=2) as pbs, \
                     tc.tile_pool(name="psb1", bufs=1, space="PSUM") as psb1:
                    ssq_pending = []

                    def rstd_half(h):
                        hsl = slice(1024 * h, 1024 * h + 1024)
                        nc.scalar.activation(out=rstd_row[0:1, hsl],
                                             in_=ssq_sb[0:1, hsl],
                                             func=Act.Ln,
                                             scale=1.0 / E,
                                             bias=eps_sb[0:1, :])
                        nc.scalar.activation(out=rstd_row[0:1, hsl],
                                             in_=rstd_row[0:1, hsl],
                                             func=Act.Exp, scale=-0.5)
                        for cc in range(2):
                            csl = slice(1024 * h + 512 * cc,
                                        1024 * h + 512 * cc + 512)
                            r_ps = psb1.tile([128, 512], f32, tag="rps",
                                             bufs=2, name=f"rps{h}_{cc}")
                            nc.tensor.matmul(r_ps, onesR_sb,
                                             rstd_row[0:1, csl], start=True,
                                             stop=True)
                            nc.scalar.activation(out=rstd_b[:, csl],
                                                 in_=r_ps, func=Act.Copy)

                    def ksumb_build():
                        # AllReduce consume. Emitted after all ssq adds so
                        # the ar_sb wait cannot block the DVE FIFO.
                        for pr in range(EI):
                            c = 64 * pr
                            nc.vector.tensor_copy(
                                out=kvbd_bf[0:64, 128 * pr:128 * pr + 64],
                                in_=ar_sb[0:64, c:c + 64])
                            nc.vector.tensor_copy(
                                out=kvbd_bf[64:128,
                                            128 * pr + 64:128 * pr + 128],
                                in_=ar_sb[64:128, c:c + 64])
                        # ksumB[:, j]: col-replicated per-head ksum via
                        # PE-transpose + block-diagonal ones matmul.
                        kst_ps = psb1.tile([8, 128], f32, tag="rps",
                                           bufs=2, name="kst")
                        nc.tensor.transpose(kst_ps, ar_sb[:, 512:520],
                                            ident_sb.bitcast(f32))
                        nc.vector.tensor_copy(out=ksumT, in_=kst_ps)
                        ksb_ps = psb1.tile([128, 512], f32, tag="rps",
                                           bufs=2, name="ksb")
                        nc.tensor.matmul(ksb_ps, ksumT, bones8,
                                         start=True, stop=True)
                        for pr in range(EI):
                            nc.vector.tensor_copy(
                                out=ksumB[0:64, pr, 0:64],
                                in_=ksb_ps[0:64, 64 * pr:64 * pr + 64])
                            nc.vector.tensor_copy(
                                out=ksumB[64:128, pr, 64:128],
                                in_=ksb_ps[64:128, 64 * pr:64 * pr + 64])

                    for sh in range(2):
                        for j in range(EI):
                            q_ps = psb1.tile([128, 1024], f32, tag="qps",
                                             bufs=2, name=f"qps{sh}_{j}")
                            for i in range(EI):
                                for sq in range(2):
                                    sl = slice(512 * sq, 512 * sq + 512)
                                    gl = slice(1024 * sh + 512 * sq,
                                               1024 * sh + 512 * sq + 512)
                                    nc.tensor.matmul(
                                        q_ps[:, sl],
                                        wq_sb[:, i, 128 * j:128 * j + 128],
                                        xq[:, i, gl],
                                        start=(i == 0), stop=(i == EI - 1))
                            # stagger ssq matmuls one block behind
                            if ssq_pending:
                                ssq_pending.pop(0)()
                            usq = pbs.tile([128, 1024], f32r, tag="usq",
                                           name=f"usq{sh}_{j}")
                            nc.scalar.activation(out=usq, in_=q_ps,
                                                 func=Act.Square,
                                                 bias=bq_sb[:, j:j + 1])
                            hs = slice(1024 * sh, 1024 * sh + 1024)
                            nc.scalar.activation(out=u_t[j][:, hs], in_=q_ps,
                                                 func=Act.Identity,
                                                 bias=bq_sb[:, j:j + 1])

                            def mk_ssq(j=j, sh=sh, usq=usq):
                                def emit():
                                    for sq in range(2):
                                        sl = slice(512 * sq, 512 * sq + 512)
                                        gl = slice(1024 * sh + 512 * sq,
                                                   1024 * sh + 512 * sq + 512)
                                        sp = psb1.tile([1, 512], f32,
                                                       tag="ssqp", bufs=2)
                                        nc.tensor.matmul(
                                            sp, onesC_sb, usq[:, sl],
                                            start=True, stop=True)
                                        nc.vector.tensor_tensor(
                                            out=ssq_sb[0:1, gl],
                                            in0=ssq_sb[0:1, gl], in1=sp,
                                            op=Alu.add)
                                return emit
                            ssq_pending.append(mk_ssq())
                            if sh == 1 and j == 1:
                                rstd_half(0)
                    for fn in ssq_pending:
                        fn()
                    rstd_half(1)
                    ksumb_build()
                    if dbg:
                        nc.sync.dma_start(out=d_ar[:, :], in_=ar_sb)
                        nc.sync.dma_start(
                            out=d_ksb[:, :],
                            in_=ksumB.rearrange("p a b -> p (a b)"))
                        nc.sync.dma_start(out=d_kvb[:, :], in_=kvbd_bf)
                        nc.sync.dma_start(out=d_rst[:, :], in_=rstd_b)
                        nc.sync.dma_start(out=d_ssq[:, :], in_=ssq_sb)
                        nc.sync.dma_start(out=d_u[:, :], in_=u_t[0])

                # ============ Phase B2 + C: divide + num + out-proj ========
                with tc.tile_pool(name="pc1", bufs=1) as pc1, \
                     tc.tile_pool(name="pb2", bufs=3) as pb2, \
                     tc.tile_pool(name="pcl", bufs=2) as pcl, \
                     tc.tile_pool(name="psb2", bufs=2, space="PSUM") as psb2:
                    wo_sb = pc1.tile([128, EI, E], f32r, tag="wo")
                    for i in range(EI):
                        nc.sync.dma_start(out=wo_sb[:, i, :],
                                          in_=woT_v[:, i, :])
                    numT = {}

                    def elu_q(g, j):
                        gsl = slice(512 * g, 512 * g + 512)
                        t1 = pb2.tile([128, 512], f32, tag="t1")
                        nc.vector.tensor_tensor(out=t1, in0=u_t[j][:, gsl],
                                                in1=rstd_b[:, gsl],
                                                op=Alu.mult)
                        ee = pb2.tile([128, 512], bf16, tag="ee2")
                        nc.scalar.activation(out=ee, in_=t1, func=Act.Exp)
                        qf = pb2.tile([128, 512], bf16, tag="qf",
                                      name=f"qf{g}_{j}")
                        nc.scalar.activation(out=qf, in_=t1, func=Act.Relu)
                        nc.vector.scalar_tensor_tensor(
                            out=qf, in0=ee, scalar=1.0, in1=qf,
                            op0=Alu.min, op1=Alu.add)
                        return qf

                    def den_num(g, j, qf):
                        gsl = slice(512 * g, 512 * g + 512)
                        den_ps = psb2.tile([128, 512], f32, tag="dps")
                        nc.tensor.matmul(den_ps, ksumB[:, j, :], qf,
                                         start=True, stop=True)
                        invd = pb2.tile([128, 512], f32, tag="invd", bufs=2)
                        nc.vector.reciprocal_approx_fast(out=invd, in_=den_ps)
                        num_ps = psb2.tile([128, 512], f32, tag="nps")
                        nc.tensor.matmul(num_ps,
                                         kvbd_bf[:, 128 * j:128 * j + 128],
                                         qf, start=True, stop=True)
                        nt = pb2.tile([128, 512], f32r, tag=f"numT{j}",
                                      bufs=2, name=f"numT{g}_{j}")
                        numT[j] = nt
                        nc.vector.scalar_tensor_tensor(
                            out=nt, in0=num_ps, scalar=1.0,
                            in1=invd, op0=Alu.mult, op1=Alu.mult)
                        if dbg and g == 0 and j == 0:
                            nc.sync.dma_start(out=d_inv[:, :], in_=invd)
                            nc.sync.dma_start(out=d_qf[:, :], in_=qf)

                    for g in range(4):
                        pend = []
                        for j in range(EI):
                            qf = elu_q(g, j)
                            pend.append((j, qf))
                            if len(pend) > 2:
                                jj, qq = pend.pop(0)
                                den_num(g, jj, qq)
                        for jj, qq in pend:
                            den_num(g, jj, qq)
                        for t4 in range(4):
                            tt = 4 * g + t4
                            tsl = slice(128 * tt, 128 * tt + 128)
                            t4sl = slice(128 * t4, 128 * t4 + 128)
                            o_ps = psb2.tile([128, E], f32, tag="ops")
                            for e in range(EI):
                                for jh in range(2):
                                    js = slice(512 * jh, 512 * jh + 512)
                                    nc.tensor.matmul(
                                        o_ps[:, js], numT[e][:, t4sl],
                                        wo_sb[:, e, js], start=(e == 0),
                                        stop=(e == EI - 1))
                            o_sb = pcl.tile([128, E], f32, tag="osb")
                            nc.vector.scalar_tensor_tensor(
                                out=o_sb, in0=o_ps, scalar=1.0,
                                in1=bo_b.bitcast(f32), op0=Alu.mult,
                                op1=Alu.add)
                            nc.sync.dma_start(out=out_d[tsl, :], in_=o_sb)

    nc.finalize()
    return nc


def _prep_inputs(inputs):
    """Host-side fold + per-core shard maps."""
    import ml_dtypes
    f = np.float32
    bf = ml_dtypes.bfloat16
    Wq, bq = inputs["Wq"], inputs["bq"]
    Wk, bk = inputs["Wk"], inputs["bk"]
    Wv, bv = inputs["Wv"], inputs["bv"]
    Wo, bo = inputs["Wo"], inputs["bo"]
    for name in ("gq", "gk"):
        assert np.allclose(np.asarray(inputs[name]), 1.0), f"{name} != 1 unsupported"
    for name in ("betaq", "betak"):
        assert np.allclose(np.asarray(inputs[name]), 0.0), f"{name} != 0 unsupported"

    wqT = np.ascontiguousarray(np.asarray(Wq, f).T)
    wqT = wqT - wqT.mean(axis=1, keepdims=True)
    bqf = np.asarray(bq, f) - np.asarray(bq, f).mean()
    wkT = np.ascontiguousarray(np.asarray(Wk, f).T)
    wkT = wkT - wkT.mean(axis=1, keepdims=True)
    bkf = np.asarray(bk, f) - np.asarray(bk, f).mean()
    wvT = np.ascontiguousarray(np.asarray(Wv, f).T)
    woT = np.ascontiguousarray(np.asarray(Wo, f).T)

    shared = {
        "wqT": np.ascontiguousarray(wqT).astype(bf),
        "wkT": np.ascontiguousarray(wkT).astype(bf),
        "wvT": wvT.astype(bf),
        "woT": woT,
        "bq2d": np.ascontiguousarray(bqf.reshape(EI, 128).T, f),
        "bkR": np.ascontiguousarray(bkf.reshape(1, E), f),
        "bvR": np.ascontiguousarray(np.asarray(bv, f).reshape(1, E)),
        "boR": np.ascontiguousarray(np.asarray(bo, f).reshape(1, E)),
        "onesR": np.ones((1, 128), f),
        "onesC": np.ones((128, 1), f),
        "ident": np.eye(128, dtype=f),
        "bones8": np.kron(np.eye(8, dtype=f), np.ones((1, 64), f)),
        "zerosBD": np.zeros((128, E), f),
    }
    qe = np.asarray(inputs["query_embed"], f)
    ke = np.asarray(inputs["key_embed"], f)
    ve = np.asarray(inputs["value"], f)
    in_maps = []
    for c in range(NCORES):
        b, hh = divmod(c, 2)
        sl = slice(hh * T, (hh + 1) * T)
        m = dict(shared)
        m["xqT"] = np.ascontiguousarray(qe[b, sl, :].T).astype(bf)
        m["xkT"] = np.ascontiguousarray(ke[b, sl, :].T).astype(bf)
        m["xvT"] = np.ascontiguousarray(ve[b, sl, :].T).astype(bf)
        in_maps.append(m)
    return in_maps


def _run(inputs, trace=False):
    from concourse.bass_utils import run_bass_kernel_spmd

    import os as _os
    dbg = bool(int(_os.environ.get("KERNEL_DBG", "0")))
    key = "nc_dbg" if dbg else "nc"
    if key not in _NC_CACHE:
        _NC_CACHE[key] = _build_nc(dbg=dbg)
    nc = _NC_CACHE[key]
    in_maps = _prep_inputs(inputs)
    res = run_bass_kernel_spmd(nc, in_maps, core_ids=list(range(NCORES)),
                               trace=trace)
    out = np.empty((B, NSEQ, E), np.float32)
    for c in range(NCORES):
        b, hh = divmod(c, 2)
        out[b, hh * T:(hh + 1) * T, :] = res.results[c]["out"]
    return out, res


def kernel(**inputs):
    out, _ = _run(inputs, trace=False)
    return out


def kernel_traced(**inputs):
    """Like kernel() but also returns (exec_time_ns, trace_path)."""
    import sys, types
    try:
        import antenv
        if "antenv.axon_hooks" not in sys.modules:
            mod = types.ModuleType("antenv.axon_hooks")
            _h = [None]
            mod.set_axon_ntff_profile_hook = lambda h: _h.__setitem__(0, h)
            mod.get_axon_ntff_profile_hook = lambda: _h[0]
            sys.modules["antenv.axon_hooks"] = mod
            antenv.axon_hooks = mod
            from trn_agent_boot.trn_boot import _ntff_profile_via_ctypes
            mod.set_axon_ntff_profile_hook(
                _ntff_profile_via_ctypes("/opt/axon/libaxon_pjrt.so"))
    except Exception as e:  # profiling is best-effort
        print(f"NTFF hook setup failed: {e}")
    out, res = _run(inputs, trace=True)
    tp = res.instructions_and_trace[1] if res.instructions_and_trace else None
    return out, res.exec_time_ns, tp


# revision 45
# speedup vs baseline: 1.6623x; 1.0200x over previous
"""MultiHeadLinearAttention Trainium2 Bass kernel — 8-core SPMD.

Problem (per reference):
  q = elu(LN(Xq @ Wq.T + bq)) + 1 ; k = elu(LN(Xk @ Wk.T + bk)) + 1
  v = Xv @ Wv.T + bv
  kv = sum_n k[n] (x) v[n]   (per head, [D,D]);  ksum = sum_n k[n]
  out = ((q @ kv) / (q . ksum + 1e-8)) @ Wo.T + bo

Sharding: core c -> batch b = c//2, token half h = c%2 (2048 q AND k/v
tokens each). Per-pair (cores 2b, 2b+1) AllReduce of kv/ksum partials
(~266 KB) completes the sum over all 4096 k/v tokens of the batch.

Pipeline design (v2):
  Phase A (k/v): per-token-tile pipeline. rstd per tile via a DVE
    rsqrt bit-trick (no Ln/Exp -> no act-table switches), elu via
    min(exp(z),1)+relu(z) (exact), k_f/v in bf16 so the kv outer-
    product matmuls run 128-wide moving in bf16. kv/ksum accumulate
    in PSUM across all 16 tiles.
  Phase B1 (q proj) emitted between kv finish and AllReduce consume so
    the collective hides under q-projection matmuls.
  Phase B2+C: per-512-token-group pipeline. den is computed directly in
    broadcast form ([128,T]) via a replicated-ksum stationary, recip on
    DVE, divide fused into the num PSUM evacuation; out-projection for
    each group follows immediately so the PE never drains.

LayerNorm mean is folded into the weights on host (W~ = W^T(I-J/E),
b~ = b - mean(b)); gq/gk==1, betaq/betak==0 (asserted).
Projections run as float32r; attention-side matmuls run bf16.
"""

import os

import numpy as np

B, NSEQ, E, H, D = 4, 4096, 1024, 16, 64
NCORES = 8
T = NSEQ // 2          # tokens per core
TT = T // 128          # token tiles (16)
EI = E // 128          # feature tiles (8)
LN_EPS = 1e-5
RSQRT_MAGIC = 0x5F3759DF

_NC_CACHE = {}


def _build_nc(dbg=False):
    from concourse import bacc
    import concourse.bass as bass
    import concourse.mybir as mybir
    import concourse.tile as tile

    f32 = mybir.dt.float32
    f32r = mybir.dt.float32r
    bf16 = mybir.dt.bfloat16
    i32 = mybir.dt.int32
    Alu = mybir.AluOpType
    Act = mybir.ActivationFunctionType
    RG = [[0, 1], [2, 3], [4, 5], [6, 7]]

    nc = bacc.Bacc(num_devices=NCORES)

    xqT = nc.dram_tensor("xqT", [E, T], bf16, kind="ExternalInput")
    xkT = nc.dram_tensor("xkT", [E, T], bf16, kind="ExternalInput")
    xvT = nc.dram_tensor("xvT", [E, T], bf16, kind="ExternalInput")
    wqT = nc.dram_tensor("wqT", [E, E], bf16, kind="ExternalInput")
    wkT = nc.dram_tensor("wkT", [E, E], bf16, kind="ExternalInput")
    wvT = nc.dram_tensor("wvT", [E, E], bf16, kind="ExternalInput")
    woT = nc.dram_tensor("woT", [E, E], f32r, kind="ExternalInput")
    bq2d = nc.dram_tensor("bq2d", [128, EI], f32, kind="ExternalInput")
    bkR = nc.dram_tensor("bkR", [1, E], f32r, kind="ExternalInput")
    bvR = nc.dram_tensor("bvR", [1, E], f32r, kind="ExternalInput")
    boR = nc.dram_tensor("boR", [1, E], f32r, kind="ExternalInput")
    onesR = nc.dram_tensor("onesR", [1, 128], f32r, kind="ExternalInput")
    onesC = nc.dram_tensor("onesC", [128, 1], f32r, kind="ExternalInput")
    ident = nc.dram_tensor("ident", [128, 128], f32r, kind="ExternalInput")
    bones8_d = nc.dram_tensor("bones8", [8, 512], f32r, kind="ExternalInput")
    zerosBD = nc.dram_tensor("zerosBD", [128, E], f32r, kind="ExternalInput")
    out_d = nc.dram_tensor("out", [T, E], f32, kind="ExternalOutput")
    if dbg:
        d_ar = nc.dram_tensor("d_ar", [128, 520], f32, kind="ExternalOutput")
        d_ksb = nc.dram_tensor("d_ksb", [128, EI * 128], bf16,
                               kind="ExternalOutput")
        d_kvb = nc.dram_tensor("d_kvb", [128, E], bf16, kind="ExternalOutput")
        d_rst = nc.dram_tensor("d_rst", [128, T], bf16, kind="ExternalOutput")
        d_ssq = nc.dram_tensor("d_ssq", [1, T], f32, kind="ExternalOutput")
        d_inv = nc.dram_tensor("d_inv", [128, 512], f32, kind="ExternalOutput")
        d_qf = nc.dram_tensor("d_qf", [128, 512], bf16, kind="ExternalOutput")
        d_u = nc.dram_tensor("d_u", [128, T], bf16, kind="ExternalOutput")

    with tile.TileContext(nc) as tc:
        with tc.tile_pool(name="const", bufs=1) as cp, \
             tc.tile_pool(name="pbq", bufs=1) as pbq, \
             tc.tile_pool(name="dram", bufs=1, space="DRAM") as dp:
            onesR_sb = cp.tile([1, 128], f32r, tag="onesR_sb")
            nc.sync.dma_start(out=onesR_sb, in_=onesR[:, :])
            onesC_sb = cp.tile([128, 1], f32r, tag="onesC_sb")
            nc.sync.dma_start(out=onesC_sb, in_=onesC[:, :])
            zrow_sb = cp.tile([1, E], f32r, tag="zrow_sb")
            nc.sync.dma_start(out=zrow_sb, in_=zerosBD[0:1, :])
            ones_col_bf = cp.tile([128, 1], bf16, tag="ones_col_bf")
            nc.vector.memset(ones_col_bf, 1.0)
            eps_sb = cp.tile([128, 1], f32, tag="eps_sb")
            nc.vector.memset(eps_sb, LN_EPS)
            bq_sb = cp.tile([128, EI], f32, tag="bq_sb")
            nc.sync.dma_start(out=bq_sb, in_=bq2d[:, :])
            bo_b = cp.tile([128, E], f32r, tag="bo_b")
            nc.sync.dma_start(out=bo_b, in_=boR[:, :].to_broadcast([128, E]))
            kvbd_bf = cp.tile([128, E], bf16, tag="kvbd_bf")
            ksumB = cp.tile([128, EI, 128], bf16, tag="ksumB")
            ksumT = cp.tile([8, 128], f32r, tag="ksumT")
            ident_sb = cp.tile([128, 128], f32r, tag="ident_sb")
            nc.sync.dma_start(out=ident_sb, in_=ident[:, :])
            bones8 = cp.tile([8, 512], f32r, tag="bones8")
            nc.sync.dma_start(out=bones8, in_=bones8_d[:, :])
            ar_sb = cp.tile([128, 520], f32, tag="ar_sb")
            cc_in = dp.tile([128, 520], f32, tag="cc_in")
            cc_out = dp.tile([128, 520], f32, tag="cc_out")

            ss_all = cp.tile([128, TT], f32, tag="ss_all")
            rs_all = cp.tile([128, TT], f32, tag="rs_all")
            rsq_s1 = cp.tile([128, TT], f32, tag="rsq_s1")
            rsq_t = cp.tile([128, TT], f32, tag="rsq_t")

            xkT_v = xkT.rearrange("(i p) n -> p i n", p=128)
            xvT_v = xvT.rearrange("(i p) n -> p i n", p=128)
            xqT_v = xqT.rearrange("(i p) n -> p i n", p=128)
            wkT_v = wkT.rearrange("(i p) j -> p i j", p=128)
            wvT_v = wvT.rearrange("(i p) j -> p i j", p=128)
            wqT_v = wqT.rearrange("(i p) j -> p i j", p=128)
            woT_v = woT.rearrange("(i p) j -> p i j", p=128)

            def rsqrt_batch(sl):
                """rs_all[:, sl] = rsqrt(ss_all[:, sl]/E + eps) via the
                inverse-sqrt bit trick + 2 Newton iterations (DVE only)."""
                s1 = rsq_s1[:, sl]
                nc.vector.tensor_scalar(out=s1, in0=ss_all[:, sl],
                                        scalar1=1.0 / E, scalar2=LN_EPS,
                                        op0=Alu.mult, op1=Alu.add)
                y = rs_all[:, sl]
                yi = y.bitcast(i32)
                nc.vector.tensor_scalar(out=yi, in0=s1.bitcast(i32),
                                        scalar1=1, scalar2=None,
                                        op0=Alu.logical_shift_right)
                nc.vector.tensor_scalar(out=yi, in0=yi,
                                        scalar1=RSQRT_MAGIC, scalar2=-1,
                                        op0=Alu.subtract, op1=Alu.mult)
                t_ = rsq_t[:, sl]
                for _ in range(2):
                    nc.vector.tensor_tensor(out=t_, in0=y, in1=y, op=Alu.mult)
                    nc.vector.tensor_tensor(out=t_, in0=t_, in1=s1,
                                            op=Alu.mult)
                    nc.vector.tensor_scalar(out=t_, in0=t_, scalar1=-0.5,
                                            scalar2=1.5, op0=Alu.mult,
                                            op1=Alu.add)
                    nc.vector.tensor_tensor(out=y, in0=y, in1=t_, op=Alu.mult)

            # ============ Phase A: k/v proj + elu + kv (16-tile pipe) ======
            xq = pbq.tile([128, EI, T], bf16, tag="xq")
            wq_sb = pbq.tile([128, EI, E], bf16, tag="wq")

            with tc.tile_pool(name="paw", bufs=1) as paw, \
                 tc.tile_pool(name="pax", bufs=5) as pax, \
                 tc.tile_pool(name="pares", bufs=1) as pares, \
                 tc.tile_pool(name="pasc", bufs=3) as pasc, \
                 tc.tile_pool(name="pskv", bufs=1, space="PSUM") as pskv, \
                 tc.tile_pool(name="psproj", bufs=2, space="PSUM") as psproj:
                wk_sb = paw.tile([128, EI, E], bf16, tag="wk")
                wv_sb = paw.tile([128, EI, E], bf16, tag="wv")
                bk_b = paw.tile([128, E], f32r, tag="bk_b")
                nc.sync.dma_start(out=bk_b,
                                  in_=bkR[:, :].to_broadcast([128, E]))
                bv_b = paw.tile([128, E], f32r, tag="bv_b")
                nc.sync.dma_start(out=bv_b,
                                  in_=bvR[:, :].to_broadcast([128, E]))

                xk_t, xv_t = {}, {}

                def load_x(t):
                    tsl = slice(128 * t, 128 * t + 128)
                    xk = pax.tile([128, EI, 128], bf16, tag="xk",
                                  name=f"xk{t}")
                    nc.sync.dma_start(out=xk, in_=xkT_v[:, :, tsl])
                    xv = pax.tile([128, EI, 128], bf16, tag="xv",
                                  name=f"xv{t}")
                    nc.sync.dma_start(out=xv, in_=xvT_v[:, :, tsl])
                    xk_t[t], xv_t[t] = xk, xv

                # first x tile, then all wk (k-proj gates on full wk),
                # then wv interleaved with the next x tiles
                load_x(0)
                for i in range(EI):
                    nc.sync.dma_start(out=wk_sb[:, i, :], in_=wkT_v[:, i, :])
                for i in range(EI):
                    nc.sync.dma_start(out=wv_sb[:, i, :], in_=wvT_v[:, i, :])
                    if i < 3:
                        load_x(i + 1)

                kv_ps = pskv.tile([128, E], f32, tag="kv_ps")
                ksum_ps = pskv.tile([128, EI], f32, tag="ksum_ps")
                for h2 in range(2):
                    nc.tensor.matmul(kv_ps[:, 512 * h2:512 * h2 + 512],
                                     onesR_sb, zrow_sb[:, 0:512], start=True,
                                     stop=False, skip_group_check=True)
                nc.tensor.matmul(ksum_ps, onesR_sb, zrow_sb[:, 0:EI],
                                 start=True, stop=False, skip_group_check=True)

                ku_t, kf_t = {}, {}

                def elu_k(t):
                    rs = rs_all[:, t:t + 1]
                    kf = pares.tile([128, E], bf16, tag=f"kf{t % 3}",
                                    name=f"kf{t}")
                    nc.scalar.activation(out=kf, in_=ku_t[t], func=Act.Relu,
                                         scale=rs)
                    ee = pasc.tile([128, E], bf16, tag="ee")
                    nc.scalar.activation(out=ee, in_=ku_t[t], func=Act.Exp,
                                         scale=rs)
                    nc.vector.scalar_tensor_tensor(
                        out=kf, in0=ee, scalar=1.0, in1=kf,
                        op0=Alu.min, op1=Alu.add)
                    kf_t[t] = kf

                def kv_mms(t):
                    kf, vu = kf_t[t], xv_t[(t, "vu")]
                    for pr in range(EI):
                        kp = kf[:, 128 * pr:128 * pr + 128]
                        nc.tensor.matmul(
                            kv_ps[:, 128 * pr:128 * pr + 128], kp,
                            vu[:, 128 * pr:128 * pr + 128], start=False,
                            stop=(t == TT - 1), skip_group_check=True)
                        nc.tensor.matmul(
                            ksum_ps[:, pr:pr + 1], kp, ones_col_bf,
                            start=False, stop=(t == TT - 1),
                            skip_group_check=True)

                for t in range(TT):
                    if t >= 3 and t + 1 < TT:
                        load_x(t + 1)
                    if 4 <= t < 4 + EI:
                        i = t - 4
                        nc.sync.dma_start(out=xq[:, i, :], in_=xqT_v[:, i, :])
                        nc.sync.dma_start(out=wq_sb[:, i, :],
                                          in_=wqT_v[:, i, :])
                    xk, xv = xk_t[t], xv_t[t]
                    k_ps = psproj.tile([128, E], f32, tag="pps",
                                       name=f"kps{t}")
                    for i in range(EI):
                        for jh in range(2):
                            js = slice(512 * jh, 512 * jh + 512)
                            nc.tensor.matmul(k_ps[:, js], xk[:, i, :],
                                             wk_sb[:, i, js], start=(i == 0),
                                             stop=(i == EI - 1))
                    ku = pares.tile([128, E], f32, tag=f"ku{t % 8}",
                                    name=f"ku{t}")
                    nc.vector.scalar_tensor_tensor(
                        out=ku, in0=k_ps, scalar=1.0, in1=bk_b.bitcast(f32),
                        op0=Alu.mult, op1=Alu.add)
                    scrap = pasc.tile([128, E], bf16, tag="scrap")
                    nc.scalar.activation(out=scrap, in_=ku, func=Act.Square,
                                         accum_out=ss_all[:, t:t + 1])
                    ku_t[t] = ku

                    v_ps = psproj.tile([128, E], f32, tag="pps",
                                       name=f"vps{t}")
                    for i in range(EI):
                        for jh in range(2):
                            js = slice(512 * jh, 512 * jh + 512)
                            nc.tensor.matmul(v_ps[:, js], xv[:, i, :],
                                             wv_sb[:, i, js], start=(i == 0),
                                             stop=(i == EI - 1))
                    vu = pares.tile([128, E], bf16, tag=f"vu{t % 8}",
                                    name=f"vu{t}")
                    nc.vector.scalar_tensor_tensor(
                        out=vu, in0=v_ps, scalar=1.0, in1=bv_b.bitcast(f32),
                        op0=Alu.mult, op1=Alu.add)
                    xv_t[(t, "vu")] = vu

                    # rsqrt per batch (finer at the tail so the last kv
                    # matmuls are not gated); elu+kv staggered two tiles
                    # behind the projections to avoid PE stalls.
                    if t in (3, 7, 11):
                        rsqrt_batch(slice(t - 3, t + 1))
                    elif t == 13:
                        rsqrt_batch(slice(12, 14))
                    elif t >= 14:
                        rsqrt_batch(slice(t, t + 1))
                    if t >= 2 and (t - 2) % 4 == 3:
                        for tb in range(t - 5, t - 1):
                            elu_k(tb)
                            kv_mms(tb)
                for tb in range(12, TT):
                    elu_k(tb)
                    kv_mms(tb)

                # pack the 16 good [64,64] blocks + ksum for the AllReduce
                pack = paw.tile([128, 520], f32, tag="pack")
                for pr in range(EI):
                    c = 64 * pr
                    nc.vector.tensor_copy(
                        out=pack[0:64, c:c + 64],
                        in_=kv_ps[0:64, 128 * pr:128 * pr + 64])
                    nc.vector.tensor_copy(
                        out=pack[64:128, c:c + 64],
                        in_=kv_ps[64:128, 128 * pr + 64:128 * pr + 128])
                nc.vector.tensor_copy(out=pack[:, 512:520], in_=ksum_ps)
                # jump the sync-queue backlog: SWDGE carries cc_in
                nc.gpsimd.dma_start(out=cc_in, in_=pack)

            nc.gpsimd.collective_compute(
                "AllReduce", Alu.add, replica_groups=RG,
                ins=[cc_in[:, :]], outs=[cc_out[:, :]])

            # ============ Phase B1: q proj + LN stats (hides AllReduce) ====
            # consume the AllReduce on the idle SWDGE queue
            nc.gpsimd.dma_start(out=ar_sb, in_=cc_out[:, :])
            nc.vector.memset(kvbd_bf, 0.0)
            nc.vector.memset(ksumB, 0.0)

            with tc.tile_pool(name="pu", bufs=1) as pu:
                u_t = [pu.tile([128, T], bf16, tag=f"u{j}", name=f"u{j}")
                       for j in range(EI)]
                rstd_row = pu.tile([1, T], f32r, tag="rstd_row")
                rstd_b = pu.tile([128, T], bf16, tag="rstd_b")
                ssq_sb = pu.tile([1, T], f32, tag="ssq_sb")
                nc.vector.memset(ssq_sb, 0.0)

                with tc.tile_pool(name="pbs", bufs=2) as pbs, \
                     tc.tile_pool(name="psb1", bufs=1, space="PSUM") as psb1:
                    ssq_pending = []

                    def rstd_half(h):
                        hsl = slice(1024 * h, 1024 * h + 1024)
                        nc.scalar.activation(out=rstd_row[0:1, hsl],
                                             in_=ssq_sb[0:1, hsl],
                                             func=Act.Ln,
                                             scale=1.0 / E,
                                             bias=eps_sb[0:1, :])
                        nc.scalar.activation(out=rstd_row[0:1, hsl],
                                             in_=rstd_row[0:1, hsl],
                                             func=Act.Exp, scale=-0.5)
                        for cc in range(2):
                            csl = slice(1024 * h + 512 * cc,
                                        1024 * h + 512 * cc + 512)
                            r_ps = psb1.tile([128, 512], f32, tag="rps",
                                             bufs=2, name=f"rps{h}_{cc}")
                            nc.tensor.matmul(r_ps, onesR_sb,
                                             rstd_row[0:1, csl], start=True,
                                             stop=True)
                            nc.scalar.activation(out=rstd_b[:, csl],
                                                 in_=r_ps, func=Act.Copy)

                    def ksumb_build():
                        # AllReduce consume. Emitted after all ssq adds so
                        # the ar_sb wait cannot block the DVE FIFO; kvbd
                        # copies go to ACT so they run parallel to ksumB
                        # (DVE) - den needs ksumB first, num needs kvbd.
                        for pr in range(EI):
                            c = 64 * pr
                            nc.scalar.copy(
                                out=kvbd_bf[0:64, 128 * pr:128 * pr + 64],
                                in_=ar_sb[0:64, c:c + 64])
                            nc.scalar.copy(
                                out=kvbd_bf[64:128,
                                            128 * pr + 64:128 * pr + 128],
                                in_=ar_sb[64:128, c:c + 64])
                        # ksumB[:, j]: col-replicated per-head ksum via
                        # PE-transpose + block-diagonal ones matmul.
                        kst_ps = psb1.tile([8, 128], f32, tag="rps",
                                           bufs=2, name="kst")
                        nc.tensor.transpose(kst_ps, ar_sb[:, 512:520],
                                            ident_sb.bitcast(f32))
                        nc.vector.tensor_copy(out=ksumT, in_=kst_ps)
                        ksb_ps = psb1.tile([128, 512], f32, tag="rps",
                                           bufs=2, name="ksb")
                        nc.tensor.matmul(ksb_ps, ksumT, bones8,
                                         start=True, stop=True)
                        for pr in range(EI):
                            nc.vector.tensor_copy(
                                out=ksumB[0:64, pr, 0:64],
                                in_=ksb_ps[0:64, 64 * pr:64 * pr + 64])
                            nc.vector.tensor_copy(
                                out=ksumB[64:128, pr, 64:128],
                                in_=ksb_ps[64:128, 64 * pr:64 * pr + 64])

                    for sh in range(2):
                        for j in range(EI):
                            q_ps = psb1.tile([128, 1024], f32, tag="qps",
                                             bufs=2, name=f"qps{sh}_{j}")
                            for i in range(EI):
                                for sq in range(2):
                                    sl = slice(512 * sq, 512 * sq + 512)
                                    gl = slice(1024 * sh + 512 * sq,
                                               1024 * sh + 512 * sq + 512)
                                    nc.tensor.matmul(
                                        q_ps[:, sl],
                                        wq_sb[:, i, 128 * j:128 * j + 128],
                                        xq[:, i, gl],
                                        start=(i == 0), stop=(i == EI - 1))
                            # stagger ssq matmuls one block behind
                            if ssq_pending:
                                ssq_pending.pop(0)()
                            usq = pbs.tile([128, 1024], f32r, tag="usq",
                                           name=f"usq{sh}_{j}")
                            nc.scalar.activation(out=usq, in_=q_ps,
                                                 func=Act.Square,
                                                 bias=bq_sb[:, j:j + 1])
                            hs = slice(1024 * sh, 1024 * sh + 1024)
                            nc.scalar.activation(out=u_t[j][:, hs], in_=q_ps,
                                                 func=Act.Identity,
                                                 bias=bq_sb[:, j:j + 1])

                            def mk_ssq(j=j, sh=sh, usq=usq):
                                def emit():
                                    for sq in range(2):
                                        sl = slice(512 * sq, 512 * sq + 512)
                                        gl = slice(1024 * sh + 512 * sq,
                                                   1024 * sh + 512 * sq + 512)
                                        sp = psb1.tile([1, 512], f32,
                                                       tag="ssqp", bufs=2)
                                        nc.tensor.matmul(
                                            sp, onesC_sb, usq[:, sl],
                                            start=True, stop=True)
                                        nc.vector.tensor_tensor(
                                            out=ssq_sb[0:1, gl],
                                            in0=ssq_sb[0:1, gl], in1=sp,
                                            op=Alu.add)
                                return emit
                            ssq_pending.append(mk_ssq())
                            if sh == 1 and j == 1:
                                rstd_half(0)
                    for fn in ssq_pending:
                        fn()
                    rstd_half(1)
                    ksumb_build()
                    if dbg:
                        nc.sync.dma_start(out=d_ar[:, :], in_=ar_sb)
                        nc.sync.dma_start(
                            out=d_ksb[:, :],
                            in_=ksumB.rearrange("p a b -> p (a b)"))
                        nc.sync.dma_start(out=d_kvb[:, :], in_=kvbd_bf)
                        nc.sync.dma_start(out=d_rst[:, :], in_=rstd_b)
                        nc.sync.dma_start(out=d_ssq[:, :], in_=ssq_sb)
                        nc.sync.dma_start(out=d_u[:, :], in_=u_t[0])

                # ============ Phase B2 + C: divide + num + out-proj ========
                with tc.tile_pool(name="pc1", bufs=1) as pc1, \
                     tc.tile_pool(name="pb2", bufs=3) as pb2, \
                     tc.tile_pool(name="pcl", bufs=2) as pcl, \
                     tc.tile_pool(name="psb2", bufs=2, space="PSUM") as psb2:
                    wo_sb = pc1.tile([128, EI, E], f32r, tag="wo")
                    for i in range(EI):
                        nc.sync.dma_start(out=wo_sb[:, i, :],
                                          in_=woT_v[:, i, :])
                    numT = {}

                    def elu_q(g, j):
                        gsl = slice(512 * g, 512 * g + 512)
                        t1 = pb2.tile([128, 512], f32, tag="t1")
                        nc.vector.tensor_tensor(out=t1, in0=u_t[j][:, gsl],
                                                in1=rstd_b[:, gsl],
                                                op=Alu.mult)
                        ee = pb2.tile([128, 512], bf16, tag="ee2")
                        nc.scalar.activation(out=ee, in_=t1, func=Act.Exp)
                        qf = pb2.tile([128, 512], bf16, tag="qf",
                                      name=f"qf{g}_{j}")
                        nc.scalar.activation(out=qf, in_=t1, func=Act.Relu)
                        nc.vector.scalar_tensor_tensor(
                            out=qf, in0=ee, scalar=1.0, in1=qf,
                            op0=Alu.min, op1=Alu.add)
                        return qf

                    def den_num(g, j, qf):
                        gsl = slice(512 * g, 512 * g + 512)
                        den_ps = psb2.tile([128, 512], f32, tag="dps")
                        nc.tensor.matmul(den_ps, ksumB[:, j, :], qf,
                                         start=True, stop=True)
                        invd = pb2.tile([128, 512], f32, tag="invd", bufs=2)
                        nc.vector.reciprocal_approx_fast(out=invd, in_=den_ps)
                        num_ps = psb2.tile([128, 512], f32, tag="nps")
                        nc.tensor.matmul(num_ps,
                                         kvbd_bf[:, 128 * j:128 * j + 128],
                                         qf, start=True, stop=True)
                        nt = pb2.tile([128, 512], f32r, tag=f"numT{j}",
                                      bufs=2, name=f"numT{g}_{j}")
                        numT[j] = nt
                        nc.vector.scalar_tensor_tensor(
                            out=nt, in0=num_ps, scalar=1.0,
                            in1=invd, op0=Alu.mult, op1=Alu.mult)
                        if dbg and g == 0 and j == 0:
                            nc.sync.dma_start(out=d_inv[:, :], in_=invd)
                            nc.sync.dma_start(out=d_qf[:, :], in_=qf)

                    for g in range(4):
                        pend = []
                        for j in range(EI):
                            qf = elu_q(g, j)
                            pend.append((j, qf))
                            if len(pend) > 2:
                                jj, qq = pend.pop(0)
                                den_num(g, jj, qq)
                        for jj, qq in pend:
                            den_num(g, jj, qq)
                        for t4 in range(4):
                            tt = 4 * g + t4
                            tsl = slice(128 * tt, 128 * tt + 128)
                            t4sl = slice(128 * t4, 128 * t4 + 128)
                            o_ps = psb2.tile([128, E], f32, tag="ops")
                            for e in range(EI):
                                for jh in range(2):
                                    js = slice(512 * jh, 512 * jh + 512)
                                    nc.tensor.matmul(
                                        o_ps[:, js], numT[e][:, t4sl],
                                        wo_sb[:, e, js], start=(e == 0),
                                        stop=(e == EI - 1))
                            o_sb = pcl.tile([128, E], f32, tag="osb")
                            nc.vector.scalar_tensor_tensor(
                                out=o_sb, in0=o_ps, scalar=1.0,
                                in1=bo_b.bitcast(f32), op0=Alu.mult,
                                op1=Alu.add)
                            nc.sync.dma_start(out=out_d[tsl, :], in_=o_sb)

    nc.finalize()
    return nc


def _prep_inputs(inputs):
    """Host-side fold + per-core shard maps."""
    import ml_dtypes
    f = np.float32
    bf = ml_dtypes.bfloat16
    Wq, bq = inputs["Wq"], inputs["bq"]
    Wk, bk = inputs["Wk"], inputs["bk"]
    Wv, bv = inputs["Wv"], inputs["bv"]
    Wo, bo = inputs["Wo"], inputs["bo"]
    for name in ("gq", "gk"):
        assert np.allclose(np.asarray(inputs[name]), 1.0), f"{name} != 1 unsupported"
    for name in ("betaq", "betak"):
        assert np.allclose(np.asarray(inputs[name]), 0.0), f"{name} != 0 unsupported"

    wqT = np.ascontiguousarray(np.asarray(Wq, f).T)
    wqT = wqT - wqT.mean(axis=1, keepdims=True)
    bqf = np.asarray(bq, f) - np.asarray(bq, f).mean()
    wkT = np.ascontiguousarray(np.asarray(Wk, f).T)
    wkT = wkT - wkT.mean(axis=1, keepdims=True)
    bkf = np.asarray(bk, f) - np.asarray(bk, f).mean()
    wvT = np.ascontiguousarray(np.asarray(Wv, f).T)
    woT = np.ascontiguousarray(np.asarray(Wo, f).T)

    shared = {
        "wqT": np.ascontiguousarray(wqT).astype(bf),
        "wkT": np.ascontiguousarray(wkT).astype(bf),
        "wvT": wvT.astype(bf),
        "woT": woT,
        "bq2d": np.ascontiguousarray(bqf.reshape(EI, 128).T, f),
        "bkR": np.ascontiguousarray(bkf.reshape(1, E), f),
        "bvR": np.ascontiguousarray(np.asarray(bv, f).reshape(1, E)),
        "boR": np.ascontiguousarray(np.asarray(bo, f).reshape(1, E)),
        "onesR": np.ones((1, 128), f),
        "onesC": np.ones((128, 1), f),
        "ident": np.eye(128, dtype=f),
        "bones8": np.kron(np.eye(8, dtype=f), np.ones((1, 64), f)),
        "zerosBD": np.zeros((128, E), f),
    }
    qe = np.asarray(inputs["query_embed"], f)
    ke = np.asarray(inputs["key_embed"], f)
    ve = np.asarray(inputs["value"], f)
    in_maps = []
    for c in range(NCORES):
        b, hh = divmod(c, 2)
        sl = slice(hh * T, (hh + 1) * T)
        m = dict(shared)
        m["xqT"] = np.ascontiguousarray(qe[b, sl, :].T).astype(bf)
        m["xkT"] = np.ascontiguousarray(ke[b, sl, :].T).astype(bf)
        m["xvT"] = np.ascontiguousarray(ve[b, sl, :].T).astype(bf)
        in_maps.append(m)
    return in_maps


def _run(inputs, trace=False):
    from concourse.bass_utils import run_bass_kernel_spmd

    import os as _os
    dbg = bool(int(_os.environ.get("KERNEL_DBG", "0")))
    key = "nc_dbg" if dbg else "nc"
    if key not in _NC_CACHE:
        _NC_CACHE[key] = _build_nc(dbg=dbg)
    nc = _NC_CACHE[key]
    in_maps = _prep_inputs(inputs)
    res = run_bass_kernel_spmd(nc, in_maps, core_ids=list(range(NCORES)),
                               trace=trace)
    out = np.empty((B, NSEQ, E), np.float32)
    for c in range(NCORES):
        b, hh = divmod(c, 2)
        out[b, hh * T:(hh + 1) * T, :] = res.results[c]["out"]
    return out, res


def kernel(**inputs):
    out, _ = _run(inputs, trace=False)
    return out


def kernel_traced(**inputs):
    """Like kernel() but also returns (exec_time_ns, trace_path)."""
    import sys, types
    try:
        import antenv
        if "antenv.axon_hooks" not in sys.modules:
            mod = types.ModuleType("antenv.axon_hooks")
            _h = [None]
            mod.set_axon_ntff_profile_hook = lambda h: _h.__setitem__(0, h)
            mod.get_axon_ntff_profile_hook = lambda: _h[0]
            sys.modules["antenv.axon_hooks"] = mod
            antenv.axon_hooks = mod
            from trn_agent_boot.trn_boot import _ntff_profile_via_ctypes
            mod.set_axon_ntff_profile_hook(
                _ntff_profile_via_ctypes("/opt/axon/libaxon_pjrt.so"))
    except Exception as e:  # profiling is best-effort
        print(f"NTFF hook setup failed: {e}")
    out, res = _run(inputs, trace=True)
    tp = res.instructions_and_trace[1] if res.instructions_and_trace else None
    return out, res.exec_time_ns, tp
